# revision 1
# baseline (speedup 1.0000x reference)
"""MixHop GNN (2 layers, 2 adjacencies, hops 0..2) on 8 trn2 NeuronCores.

Sharding: nodes row-partitioned across 8 cores (6250 rows each). Each SpMM
is computed for the core's destination rows only, gathering source rows
from a replicated DRAM table (AllGather between phases). The SpMM maps to
TensorE as a segment matmul: gather 128-edge tiles of source rows, build a
val-scaled one-hot matrix M[e, d] = val[e] * (lrow[e] == d) on the vector
engine, and accumulate M^T @ G into a PSUM block of 128 destination rows.

v2: gathers are BATCHED via nc.gpsimd.dma_gather (DMAGatherAnt) — one call
covers a whole chunk of blocks instead of one 128-edge tile, cutting the
SWDGE descriptor-prep cost (994ns fixed per call) by ~40x. Because gather
indices are int16, tables are split at row 32768 (L/H halves, one call
each); pad slots use index 0 with val 0. Layer-1 linear biases are skipped
(BatchNorm subtracts the per-feature mean, so constant per-feature shifts
cancel exactly). BN scale/shift+relu are fused into the scalar engine's
activation op applied to PE-transposed (feature-major) tiles.

Self-contained: only numpy + ml_dtypes + concourse (environment packages).
"""
import numpy as np

import concourse.bass as bass
from concourse import mybir
from concourse.bass_utils import run_bass_kernel_spmd
from concourse.library_config import mlp
from concourse.tile import TileContext

F32 = mybir.dt.float32
BF16 = mybir.dt.bfloat16
I16 = mybir.dt.int16
AL = mybir.AluOpType
AF = mybir.ActivationFunctionType

N = 50000
NCORES = 8
R = N // NCORES          # 6250 rows per core
BLK = 128
NB = (R + BLK - 1) // BLK  # 49 blocks (48 full + 106)
LAM = 0.5
EPS = 1e-5
P = 128
HALF = 32768             # int16 gather-index limit
WIN = 8                  # 128-edge tiles per dma_gather call (1024 idxs =
                         # the SWDGE descriptor-ring per-call capacity)

TRACE = False            # set by test harness for profiling runs
FIX_WAITS = True         # disable for CoreSim validation (sim rejects
                         # post-hoc sync_info edits)
STOP_AFTER = None        # debug: truncate pipeline after a named stage
_STAGES = ['y12', 'hop0', 'a1', 'n1', 'a2', 'n2', 'bn', 'dense',
           'a3', 'n3', 'a4', 'n4']


def _on(stage):
    if STOP_AFTER is None:
        return True
    return _STAGES.index(stage) <= _STAGES.index(STOP_AFTER)
LAST_RESULT = {}


# ---------------------------------------------------------------- BIR post-pass
ASYNC_OPCODES = {"DMACopy", "CollectiveCompute", "DMAGatherAnt",
                 "DMAScatterAddAnt", "DMATransposeAnt"}


def _cap(inst) -> int:
    if inst.opcode in ("EventSemaphore", "NoOp"):
        return 999
    return 1


def fix_waits(nc, verbose=False):
    # --- collect streams (blocks concatenated in listed order; Tile output
    # is straight-line per engine)
    all_bbs = [bb for fn in nc.m.functions for bb in fn.blocks]
    streams = {}
    for bb in all_bbs:
        for inst in bb.instructions:
            streams.setdefault(inst.engine, []).append(inst)

    unsafe = set()
    wait_list = {}
    upd_list = {}
    for eng, insts in streams.items():
        for inst in insts:
            si = inst.sync_info
            ws, us = [], []
            if si:
                for w in (si.on_wait or []):
                    if getattr(w, "wait_mode", None) == "sem-ge-imm" and isinstance(
                            getattr(w, "wait_value", None), int):
                        ws.append((w.id, w.wait_value, w))
                    else:
                        ws.append((w.id, None, w))
                        unsafe.add(w.id)
                for u in (si.on_update or []):
                    um = getattr(u, "update_mode", None)
                    uv = getattr(u, "update_value", None)
                    if um == "sem-add-imm" and isinstance(uv, int):
                        us.append((u.id, uv))
                    elif um == "sem-inc":
                        us.append((u.id, 1))
                    else:
                        us.append((u.id, 0))
                        unsafe.add(u.id)
            wait_list[id(inst)] = ws
            upd_list[id(inst)] = us

    engines = list(streams.keys())
    ptr = {e: 0 for e in engines}
    vc = {e: {} for e in engines}
    sem_level = {}
    sem_cums = {}
    sem_snaps = {}

    def knowledge(s, v):
        cums = sem_cums.get(s)
        if not cums:
            return None
        import bisect
        i = bisect.bisect_left(cums, v)
        if i >= len(cums):
            i = len(cums) - 1
        return sem_snaps[s][i]

    n_dropped = 0
    progressed = True
    while progressed:
        progressed = False
        for eng in engines:
            insts = streams[eng]
            while ptr[eng] < len(insts):
                inst = insts[ptr[eng]]
                ws = wait_list[id(inst)]
                ok = True
                for (s, v, w) in ws:
                    if s in unsafe or v is None:
                        continue
                    if sem_level.get(s, 0) < v:
                        ok = False
                        break
                if not ok:
                    break
                myvc = vc[eng]
                kept = []
                # engine sems first: their knowledge usually implies the
                # DMA-lane waits, letting us drop the latter
                ws = sorted(ws, key=lambda t: str(
                    getattr(t[2], "ant_name", "")).startswith("DMA"))
                for (s, v, w) in ws:
                    if s not in unsafe and v is not None and myvc.get(s, 0) >= v:
                        n_dropped += 1
                        continue
                    kept.append(w)
                    if s in unsafe or v is None:
                        continue
                    k = knowledge(s, v)
                    if k:
                        for ks, kv in k.items():
                            if myvc.get(ks, 0) < kv:
                                myvc[ks] = kv
                    if myvc.get(s, 0) < v:
                        myvc[s] = v
                si = inst.sync_info
                if si and len(kept) != len(si.on_wait or []):
                    inst.sync_info = mybir.SyncInfo(
                        on_wait=kept, on_update=list(si.on_update or []))
                us = upd_list[id(inst)]
                if us:
                    is_async = inst.opcode in ASYNC_OPCODES
                    for (s, u) in us:
                        lvl = sem_level.get(s, 0) + u
                        sem_level[s] = lvl
                        if s not in unsafe:
                            snap = dict(myvc)
                            snap[s] = lvl
                            cums = sem_cums.setdefault(s, [])
                            snaps = sem_snaps.setdefault(s, [])
                            if snaps:
                                prev = snaps[-1]
                                for ks, kv in prev.items():
                                    if snap.get(ks, 0) < kv:
                                        snap[ks] = kv
                            cums.append(lvl)
                            snaps.append(snap)
                            if not is_async:
                                myvc[s] = lvl
                ptr[eng] += 1
                progressed = True

    stuck = sum(len(streams[e]) - ptr[e] for e in engines)
    # --- cap remaining waits with carriers
    uid = 0
    n_carriers = 0
    for bb in all_bbs:
        new_insts = []
        for inst in bb.instructions:
            si = inst.sync_info
            waits = list(si.on_wait) if (si and si.on_wait) else []
            cap = _cap(inst)
            if len(waits) > cap:
                keep = waits[len(waits) - cap:]
                for w in waits[: len(waits) - cap]:
                    uid += 1
                    new_insts.append(mybir.InstEventSemaphore(
                        name=f"waitfix-{uid}",
                        engine=inst.engine, ins=[], outs=[],
                        sync_info=mybir.SyncInfo(on_wait=[w], on_update=[]),
                    ))
                    n_carriers += 1
                inst.sync_info = mybir.SyncInfo(
                    on_wait=keep, on_update=list(si.on_update or []))
            new_insts.append(inst)
        bb.instructions = new_insts
    if verbose:
        print(f"fix_waits: dropped {n_dropped} redundant waits, "
              f"{n_carriers} carriers, {stuck} unprocessed")
    return nc


# ---------------------------------------------------------------- preprocessing
def _prep_adj(row, col, val):
    """Partition edges by destination core / 128-row block, then split each
    block's edges by source half (col < HALF vs >=) for int16 gather
    indices. Tile layout per pass: [all blocks' L tiles][all blocks' H
    tiles]; gather calls cover WIN-tile windows of each region.

    Returns:
      idx16 [NCORES, 128, T*8] int16  gather indices (wrapped 16-partition
                                      layout, replicated 8x)
      lrowv [NCORES, 128, T] f32      local dest row per edge slot
      valv  [NCORES, 128, T] f32      edge weight per slot (0 = pad)
      sched (TL, TH, [(b, lt0, kL, ht0, kH), ...])
      T = TL + TH total 128-edge tiles
    """
    row = np.asarray(row); col = np.asarray(col); val = np.asarray(val)
    core = row // R
    rloc = row - core * R
    blk = rloc // BLK
    lrow = rloc - blk * BLK
    half = (col >= HALF).astype(np.int64)
    idxval = np.where(half == 1, col - HALF, col).astype(np.int64)

    cnt = np.zeros((NCORES, NB, 2), np.int64)
    np.add.at(cnt, (core, blk, half), 1)
    kL = np.maximum(1, -(-cnt[:, :, 0].max(axis=0) // BLK))  # [NB]
    kH = np.maximum(1, -(-cnt[:, :, 1].max(axis=0) // BLK))  # [NB]

    lt0 = np.concatenate([[0], np.cumsum(kL)])
    TL = int(lt0[-1])
    ht0 = TL + np.concatenate([[0], np.cumsum(kH)])
    T = int(ht0[-1])
    sched = (TL, T - TL,
             [(b, int(lt0[b]), int(kL[b]), int(ht0[b]), int(kH[b]))
              for b in range(NB)])

    idx_flat = np.zeros((NCORES, T * BLK), np.int64)
    lrow_flat = np.zeros((NCORES, T * BLK), np.float32)
    val_flat = np.zeros((NCORES, T * BLK), np.float32)

    order = np.lexsort((col, half, blk, core))
    core_s, blk_s, half_s = core[order], blk[order], half[order]
    idx_s, lrow_s, val_s = idxval[order], lrow[order], val[order]
    key = (core_s * NB + blk_s) * 2 + half_s
    grid = np.arange(NCORES * NB * 2)
    starts = np.searchsorted(key, grid)
    ends = np.searchsorted(key, grid + 1)
    for c in range(NCORES):
        for b in range(NB):
            for h, base in ((0, lt0[b]), (1, ht0[b])):
                g = (c * NB + b) * 2 + h
                s, e = starts[g], ends[g]
                n = e - s
                if n == 0:
                    continue
                off = int(base) * BLK
                idx_flat[c, off:off + n] = idx_s[s:e]
                lrow_flat[c, off:off + n] = lrow_s[s:e]
                val_flat[c, off:off + n] = val_s[s:e]

    # idx wrap: call-local position i = t*128 + p lives at partition i%16,
    # column i//16 = t*8 + p//16 (call starts are tile-aligned, 128%16==0).
    arr = idx_flat.reshape(NCORES, T, 8, 16)           # [c, t, p//16, p%16]
    idx16 = arr.transpose(0, 3, 1, 2).reshape(NCORES, 16, T * 8)
    idx16 = np.ascontiguousarray(
        np.tile(idx16, (1, 8, 1)).astype(np.int16))    # [c, 128, T*8]
    lrowv = np.ascontiguousarray(
        lrow_flat.reshape(NCORES, T, BLK).transpose(0, 2, 1))
    valv = np.ascontiguousarray(
        val_flat.reshape(NCORES, T, BLK).transpose(0, 2, 1))
    return idx16, lrowv, valv, sched, T


# ---------------------------------------------------------------- device program
def _build(TA, TN, sched_a, sched_n):
    nc = bass.Bass(num_devices=NCORES)
    groups = [list(range(NCORES))]

    # ---- external I/O
    xl_in = nc.declare_dram_parameter("xl", [P, R], BF16, isOutput=False)
    a_idx = nc.declare_dram_parameter("a_idx", [P, TA * 8], I16, isOutput=False)
    a_lrow = nc.declare_dram_parameter("a_lrow", [P, TA], F32, isOutput=False)
    a_val = nc.declare_dram_parameter("a_val", [P, TA], F32, isOutput=False)
    n_idx = nc.declare_dram_parameter("n_idx", [P, TN * 8], I16, isOutput=False)
    n_lrow = nc.declare_dram_parameter("n_lrow", [P, TN], F32, isOutput=False)
    n_val = nc.declare_dram_parameter("n_val", [P, TN], F32, isOutput=False)
    iota_in = nc.declare_dram_parameter("iota", [P, P], F32, isOutput=False)
    ident_in = nc.declare_dram_parameter("ident", [P, P], BF16, isOutput=False)
    ones1_in = nc.declare_dram_parameter("ones1", [1, P], F32, isOutput=False)
    onesb_in = nc.declare_dram_parameter("onesb", [P, 1], BF16, isOutput=False)
    w1h_in = nc.declare_dram_parameter("w1h", [128, 256], BF16, isOutput=False)
    w10_in = nc.declare_dram_parameter("w10", [128, 128], BF16, isOutput=False)
    w2h_in = nc.declare_dram_parameter("w2h", [384, 256], BF16, isOutput=False)
    w20_in = nc.declare_dram_parameter("w20", [384, 128], BF16, isOutput=False)
    fpw_in = nc.declare_dram_parameter("fpw", [384, 128], BF16, isOutput=False)
    b2h_in = nc.declare_dram_parameter("b2h", [1, 256], F32, isOutput=False)
    b20_in = nc.declare_dram_parameter("b20", [1, 128], F32, isOutput=False)
    fpb_in = nc.declare_dram_parameter("fpb", [1, 128], F32, isOutput=False)
    bng_in = nc.declare_dram_parameter("bng", [1, 384], F32, isOutput=False)
    bnb_in = nc.declare_dram_parameter("bnb", [1, 384], F32, isOutput=False)
    out_ext = nc.declare_dram_parameter("out", [R, 128], F32, isOutput=True)

    with TileContext(nc) as tc:
        with (
            tc.tile_pool(name="consts", bufs=1) as consts,
            tc.tile_pool(name="glp", bufs=4) as glp,
            tc.tile_pool(name="mp", bufs=8) as mp,
            tc.tile_pool(name="wk", bufs=4) as wk,
            tc.tile_pool(name="ps", bufs=3, space="PSUM") as ps,
            tc.tile_pool(name="pm", bufs=1, space="PSUM") as pm,
            tc.tile_pool(name="ptr", bufs=2, space="PSUM") as ptrp,
            tc.tile_pool(name="pst", bufs=1, space="PSUM") as pstp,
            tc.tile_pool(name="dram", bufs=1, space="DRAM") as dram,
        ):
            nc.gpsimd.load_library(mlp)

            # num_idxs register cache: Pool has ~46 free registers and
            # to_reg() does not dedupe constants
            _regs = {}

            def nreg(v):
                if v not in _regs:
                    _regs[v] = nc.gpsimd.to_reg(v)
                return _regs[v]

            # ---------------- DRAM scratch
            y12_loc = dram.tile([R, 256], BF16)
            ta_loc = dram.tile([R, 128], BF16)
            tn_loc = dram.tile([R, 128], BF16)
            z12_loc = dram.tile([R, 256], BF16)
            tpa_loc = dram.tile([R, 128], BF16)
            tpn_loc = dram.tile([R, 128], BF16)
            bn_loc = dram.tile([1, 768], F32)
            ss_dram = dram.tile([6, 128], F32)
            y12_full = dram.tile([N, 256], BF16, addr_space="Shared")
            ta_full = dram.tile([N, 128], BF16, addr_space="Shared")
            tn_full = dram.tile([N, 128], BF16, addr_space="Shared")
            z12_full = dram.tile([N, 256], BF16, addr_space="Shared")
            tpa_full = dram.tile([N, 128], BF16, addr_space="Shared")
            tpn_full = dram.tile([N, 128], BF16, addr_space="Shared")
            bn_full = dram.tile([1, 768], F32, addr_space="Shared")

            # ---------------- constants to SBUF
            def cload(src, shape, dtype):
                t = consts.tile(shape, dtype, name=f"c_{src.name}")
                nc.sync.dma_start(out=t[:], in_=src[:])
                return t

            xl = cload(xl_in, [P, R], BF16)
            iota = cload(iota_in, [P, P], F32)
            identb = cload(ident_in, [P, P], BF16)
            ones1 = cload(ones1_in, [1, P], F32)
            onesb = cload(onesb_in, [P, 1], BF16)
            w1h = cload(w1h_in, [128, 256], BF16)
            w10 = cload(w10_in, [128, 128], BF16)
            b2h = cload(b2h_in, [1, 256], F32)
            b20 = cload(b20_in, [1, 128], F32)
            fpb = cload(fpb_in, [1, 128], F32)
            bng = cload(bng_in, [1, 384], F32)
            bnb = cload(bnb_in, [1, 384], F32)
            w2h_sb, w20_sb, fpw_sb = [], [], []
            for k in range(3):
                t = consts.tile([128, 256], BF16, name=f"w2h{k}")
                nc.sync.dma_start(out=t[:], in_=w2h_in[k * 128:(k + 1) * 128, :])
                w2h_sb.append(t)
                t = consts.tile([128, 128], BF16, name=f"w20{k}")
                nc.sync.dma_start(out=t[:], in_=w20_in[k * 128:(k + 1) * 128, :])
                w20_sb.append(t)
                t = consts.tile([128, 128], BF16, name=f"fpw{k}")
                nc.sync.dma_start(out=t[:], in_=fpw_in[k * 128:(k + 1) * 128, :])
                fpw_sb.append(t)

            aidx = cload(a_idx, [P, TA * 8], I16)
            alrow = cload(a_lrow, [P, TA], F32)
            aval = cload(a_val, [P, TA], F32)
            nidx = cload(n_idx, [P, TN * 8], I16)
            nlrow = cload(n_lrow, [P, TN], F32)
            nval = cload(n_val, [P, TN], F32)

            # persistent per-node-block SBUF arrays. Layer-2's three hop
            # slices reuse h1_all's storage: block b's layer-1 features die
            # exactly when its layer-2 values are produced (hop0' is written
            # after the stage-7 transposes read the block; hop1'/hop2' are
            # written in later passes).
            h1_all = consts.tile([P, NB * 384], BF16, name="h1_all")
            sclT = consts.tile([P, 3], F32, name="sclT")
            shfT = consts.tile([P, 3], F32, name="shfT")

            def h2a(b, nb=P):
                return h1_all[:nb, b * 384:b * 384 + 128]

            def h2b(b, nb=P):
                return h1_all[:nb, b * 384 + 128:b * 384 + 256]

            def h2c(b, nb=P):
                return h1_all[:nb, b * 384 + 256:b * 384 + 384]

            def nb_of(b):
                return BLK if b < NB - 1 else R - BLK * (NB - 1)

            # broadcast bias rows to all partitions (b2h, b20*1.5, fpb)
            bcast = {}
            for nm, bsrc, wdt in (("b2h", b2h, 256), ("b20", b20, 128),
                                  ("fpb", fpb, 128)):
                pbx = pm.tile([P, 256], F32, tag="pmm", name=f"pb_{nm}")
                nc.tensor.matmul(out=pbx[:, :wdt], lhsT=ones1[:], rhs=bsrc[:],
                                 start=True, stop=True)
                bt = consts.tile([P, wdt], F32, name=f"bb_{nm}")
                nc.vector.tensor_copy(out=bt[:], in_=pbx[:, :wdt])
                bcast[nm] = bt

            # ---------------- stage 1: local Y12 = X @ [W1;W2]^T (no bias:
            # layer-1 biases cancel in BatchNorm), then AllGather.
            for b in range(NB) if _on('y12') else []:
                nb = nb_of(b)
                py = pm.tile([P, 256], F32, tag="pmm")
                nc.tensor.matmul(out=py[:nb, :], lhsT=xl[:, b * BLK:b * BLK + nb],
                                 rhs=w1h[:], start=True, stop=True)
                yb = wk.tile([P, 256], BF16, tag="yb")
                nc.scalar.activation(out=yb[:nb, :], in_=py[:nb, :], func=AF.Copy)
                nc.sync.dma_start(out=y12_loc[b * BLK:b * BLK + nb, :],
                                  in_=yb[:nb, :])
            if _on('y12'):
                nc.gpsimd.collective_compute(
                    "AllGather", AL.bypass, replica_groups=groups,
                    ins=[y12_loc[:]], outs=[y12_full[:]])

            # hop0 while the AllGather runs: h1 hop0 = 1.5 * x @ W0^T
            # (w10 pre-scaled by 1.5 on host; bias cancels in BN)
            for b in range(NB) if _on('hop0') else []:
                nb = nb_of(b)
                p0 = pm.tile([P, 256], F32, tag="pmm")
                nc.tensor.matmul(out=p0[:nb, :128], lhsT=xl[:, b * BLK:b * BLK + nb],
                                 rhs=w10[:], start=True, stop=True)
                nc.scalar.activation(out=h1_all[:nb, b * 384:b * 384 + 128],
                                     in_=p0[:nb, :128], func=AF.Copy)

            # ---------------- batched-gather SpMM pass: L phase over the
            # low table half, then H phase over the high half (int16 gather
            # indices). Gather calls cover WIN-tile windows: the SWDGE
            # descriptor ring caps one call at WIN*128 descriptors. Each
            # block accumulates one PSUM per phase; the H drain combines
            # with the L result.
            tstage = consts.tile([P, NB * 128], BF16, name="tstage")

            def spmm_phase(pfx, table_half, idxT, lrowT, valT, t0, ntiles,
                           binfo, elem, drain):
                wins = []
                for w0 in range(0, ntiles, WIN):
                    n = min(WIN, ntiles - w0)
                    g = glp.tile([P, WIN * elem], BF16, tag="gw",
                                 name=f"{pfx}g{w0}")
                    nc.gpsimd.dma_gather(
                        out_ap=g[:, :n * elem].rearrange(
                            "p (t e) -> p t e", e=elem),
                        in_ap=table_half,
                        idxs_ap=idxT[:, (t0 + w0) * 8:(t0 + w0 + n) * 8],
                        num_idxs=n * BLK, num_idxs_reg=nreg(n * BLK),
                        elem_size=elem)
                    wins.append(g)
                for (b, bt0, k) in binfo:
                    psum = ps.tile([P, elem], F32, tag="sp", name=f"{pfx}ps{b}")
                    for i in range(k):
                        t = bt0 + i
                        m = mp.tile([P, P], BF16, tag="m")
                        nc.vector.tensor_scalar(
                            out=m[:], in0=iota[:],
                            scalar1=lrowT[:, t:t + 1],
                            scalar2=valT[:, t:t + 1],
                            op0=AL.is_equal, op1=AL.mult)
                        lt = t - t0
                        g = wins[lt // WIN]
                        s = lt % WIN
                        nc.tensor.matmul(
                            out=psum[:], lhsT=m[:],
                            rhs=g[:, s * elem:(s + 1) * elem],
                            start=(i == 0), stop=(i == k - 1))
                    drain(b, psum)

            def spmm_pass(pfx, table, idxT, lrowT, valT, sched, elem,
                          drainL, drainH):
                TL, TH, blocks = sched
                spmm_phase(pfx + "L", table[0:HALF, :], idxT, lrowT, valT,
                           0, TL,
                           [(b, lt0, kL) for (b, lt0, kL, _, _) in blocks],
                           elem, drainL)
                spmm_phase(pfx + "H", table[HALF:N, :], idxT, lrowT, valT,
                           TL, TH,
                           [(b, ht0, kH) for (b, _, _, ht0, kH) in blocks],
                           elem, drainH)

            # ---------------- layer 1, first application (hop1 + T tables)
            def mk_first_app(hslice, lam_first, t_loc):
                # hslice(b) -> target AP for the hop-1 slice; lam_first:
                # None for the A pass (copy), LAM for the ND pass (l-add)
                def dL(b, p):
                    sl = hslice(b)
                    if lam_first is None:
                        nc.vector.tensor_copy(out=sl, in_=p[:, 0:128])
                    else:
                        nc.vector.scalar_tensor_tensor(
                            out=sl, in0=p[:, 0:128], scalar=lam_first, in1=sl,
                            op0=AL.mult, op1=AL.add)
                    nc.scalar.activation(out=tstage[:, b * 128:(b + 1) * 128],
                                         in_=p[:, 128:256], func=AF.Copy)

                def dH(b, p):
                    nb = nb_of(b)
                    sl = hslice(b)
                    if lam_first is None:
                        nc.vector.tensor_tensor(out=sl, in0=p[:, 0:128],
                                                in1=sl, op=AL.add)
                    else:
                        nc.vector.scalar_tensor_tensor(
                            out=sl, in0=p[:, 0:128], scalar=lam_first, in1=sl,
                            op0=AL.mult, op1=AL.add)
                    tsb = wk.tile([P, 128], BF16, tag="tsb")
                    nc.vector.tensor_tensor(
                        out=tsb[:], in0=p[:, 128:256],
                        in1=tstage[:, b * 128:(b + 1) * 128], op=AL.add)
                    nc.sync.dma_start(out=t_loc[b * BLK:b * BLK + nb, :],
                                      in_=tsb[:nb, :])
                return dL, dH

            def h1_hop1(b, nb=P):
                return h1_all[:nb, b * 384 + 128:b * 384 + 256]

            def h1_hop2(b, nb=P):
                return h1_all[:nb, b * 384 + 256:b * 384 + 384]

            if _on('a1'):
                dL, dH = mk_first_app(h1_hop1, None, ta_loc)
                spmm_pass("a1", y12_full, aidx, alrow, aval, sched_a, 256,
                          dL, dH)
                nc.gpsimd.collective_compute(
                    "AllGather", AL.bypass, replica_groups=groups,
                    ins=[ta_loc[:]], outs=[ta_full[:]])
            if _on('n1'):
                dL, dH = mk_first_app(h1_hop1, LAM, tn_loc)
                spmm_pass("n1", y12_full, nidx, nlrow, nval, sched_n, 256,
                          dL, dH)
                nc.gpsimd.collective_compute(
                    "AllGather", AL.bypass, replica_groups=groups,
                    ins=[tn_loc[:]], outs=[tn_full[:]])

            # ---------------- layer 1, second application (hop2) + BN stats
            def mk_second_app(hslice, lam, extra=None):
                def dL(b, p):
                    sl = hslice(b)
                    if lam is None:
                        nc.vector.tensor_copy(out=sl, in_=p[:, 0:128])
                    else:
                        nc.vector.scalar_tensor_tensor(
                            out=sl, in0=p[:, 0:128], scalar=lam, in1=sl,
                            op0=AL.mult, op1=AL.add)

                def dH(b, p):
                    sl = hslice(b)
                    if lam is None:
                        nc.vector.tensor_tensor(out=sl, in0=p[:, 0:128],
                                                in1=sl, op=AL.add)
                    else:
                        nc.vector.scalar_tensor_tensor(
                            out=sl, in0=p[:, 0:128], scalar=lam, in1=sl,
                            op0=AL.mult, op1=AL.add)
                    if extra is not None:
                        extra(b)
                return dL, dH

            if _on('a2'):
                dL, dH = mk_second_app(h1_hop2, None)
                spmm_pass("a2", ta_full, aidx, alrow, aval, sched_a, 128,
                          dL, dH)

            pst = pstp.tile([1, 384], F32, tag="pst", name="pst")
            psq = pstp.tile([1, 384], F32, tag="psq", name="psq")

            def stats_extra(b):
                nb = nb_of(b)
                hsl = h1_all[:, b * 384:b * 384 + 384]
                sq = wk.tile([P, 384], BF16, tag="sq")
                nc.vector.tensor_tensor(out=sq[:nb, :], in0=hsl[:nb],
                                        in1=hsl[:nb], op=AL.mult)
                nc.tensor.matmul(out=pst[:], lhsT=onesb[:nb, :], rhs=hsl[:nb],
                                 start=(b == 0), stop=(b == NB - 1))
                nc.tensor.matmul(out=psq[:], lhsT=onesb[:nb, :], rhs=sq[:nb, :],
                                 start=(b == 0), stop=(b == NB - 1))

            if _on('n2'):
                dL, dH = mk_second_app(h1_hop2, LAM, extra=stats_extra)
                spmm_pass("n2", tn_full, nidx, nlrow, nval, sched_n, 128,
                          dL, dH)

            # ---------------- BN finalize (allreduce + feature-major params)
            if _on('bn'):
                stats = wk.tile([1, 768], F32, tag="bnst", bufs=1)
                nc.vector.tensor_copy(out=stats[:, 0:384], in_=pst[:])
                nc.vector.tensor_copy(out=stats[:, 384:768], in_=psq[:])
                nc.sync.dma_start(out=bn_loc[:], in_=stats[:])
                nc.gpsimd.collective_compute(
                    "AllReduce", AL.add, replica_groups=groups,
                    ins=[bn_loc[:]], outs=[bn_full[:]])
                bnr = wk.tile([1, 768], F32, tag="bnr", bufs=1)
                nc.sync.dma_start(out=bnr[:], in_=bn_full[:])
                mean = wk.tile([1, 384], F32, tag="bn1", bufs=1)
                var = wk.tile([1, 384], F32, tag="bn2", bufs=1)
                tmp = wk.tile([1, 384], F32, tag="bn3", bufs=1)
                sshf = wk.tile([1, 768], F32, tag="bn4", bufs=1)
                nc.vector.tensor_scalar(out=mean[:], in0=bnr[:, 0:384],
                                        scalar1=1.0 / N, scalar2=None, op0=AL.mult)
                nc.vector.tensor_scalar(out=var[:], in0=bnr[:, 384:768],
                                        scalar1=1.0 / N, scalar2=None, op0=AL.mult)
                nc.vector.tensor_tensor(out=tmp[:], in0=mean[:], in1=mean[:], op=AL.mult)
                nc.vector.tensor_tensor(out=var[:], in0=var[:], in1=tmp[:], op=AL.subtract)
                nc.vector.tensor_scalar(out=var[:], in0=var[:], scalar1=EPS,
                                        scalar2=None, op0=AL.add)
                nc.scalar.sqrt(out=var[:], in_=var[:])
                nc.vector.reciprocal(out=var[:], in_=var[:])
                nc.vector.tensor_tensor(out=sshf[:, 0:384], in0=bng[:], in1=var[:],
                                        op=AL.mult)
                nc.vector.tensor_tensor(out=tmp[:], in0=mean[:], in1=sshf[:, 0:384],
                                        op=AL.mult)
                nc.vector.tensor_tensor(out=sshf[:, 384:768], in0=bnb[:], in1=tmp[:],
                                        op=AL.subtract)
                nc.sync.dma_start(out=ss_dram[:], in_=sshf[:])
                for k in range(3):
                    nc.sync.dma_start(out=sclT[:, k:k + 1], in_=ss_dram[k:k + 1, :])
                    nc.sync.dma_start(out=shfT[:, k:k + 1], in_=ss_dram[k + 3:k + 4, :])

            # ---------------- layer-2 dense: transpose, fused BN+relu (ACT),
            # linears; z12 local then AllGather
            for b in range(NB) if _on('dense') else []:
                nb = nb_of(b)
                hbT = wk.tile([P, 384], BF16, tag="hbT")
                for k in range(3):
                    pt = ptrp.tile([P, 128], BF16, tag="pt")
                    nc.tensor.transpose(
                        out=pt[:, :nb],
                        in_=h1_all[:nb, b * 384 + k * 128:b * 384 + (k + 1) * 128],
                        identity=identb[:nb, :nb])
                    nc.scalar.activation(
                        out=hbT[:, k * 128:k * 128 + nb], in_=pt[:, :nb],
                        func=AF.Relu, scale=sclT[:, k:k + 1], bias=shfT[:, k:k + 1])
                pz = pm.tile([P, 256], F32, tag="pmm")
                for k in range(3):
                    nc.tensor.matmul(out=pz[:nb, :],
                                     lhsT=hbT[:, k * 128:k * 128 + nb],
                                     rhs=w2h_sb[k][:], start=(k == 0), stop=(k == 2))
                zt = wk.tile([P, 256], BF16, tag="yb")
                nc.vector.tensor_tensor(out=zt[:nb, :], in0=pz[:nb, :],
                                        in1=bcast["b2h"][:nb, :], op=AL.add)
                nc.sync.dma_start(out=z12_loc[b * BLK:b * BLK + nb, :], in_=zt[:nb, :])
                p0 = pm.tile([P, 256], F32, tag="pmm")
                for k in range(3):
                    nc.tensor.matmul(out=p0[:nb, :128],
                                     lhsT=hbT[:, k * 128:k * 128 + nb],
                                     rhs=w20_sb[k][:], start=(k == 0), stop=(k == 2))
                nc.vector.tensor_tensor(out=h2a(b, nb),
                                        in0=p0[:nb, :128], in1=bcast["b20"][:nb, :],
                                        op=AL.add)
            if _on('dense'):
                nc.gpsimd.collective_compute(
                    "AllGather", AL.bypass, replica_groups=groups,
                    ins=[z12_loc[:]], outs=[z12_full[:]])

            # ---------------- layer 2, first + second applications
            if _on('a3'):
                dL, dH = mk_first_app(h2b, None, tpa_loc)
                spmm_pass("a3", z12_full, aidx, alrow, aval, sched_a, 256,
                          dL, dH)
                nc.gpsimd.collective_compute(
                    "AllGather", AL.bypass, replica_groups=groups,
                    ins=[tpa_loc[:]], outs=[tpa_full[:]])
            if _on('n3'):
                dL, dH = mk_first_app(h2b, LAM, tpn_loc)
                spmm_pass("n3", z12_full, nidx, nlrow, nval, sched_n, 256,
                          dL, dH)
                nc.gpsimd.collective_compute(
                    "AllGather", AL.bypass, replica_groups=groups,
                    ins=[tpn_loc[:]], outs=[tpn_full[:]])

            if _on('a4'):
                dL, dH = mk_second_app(h2c, None)
                spmm_pass("a4", tpa_full, aidx, alrow, aval, sched_a, 128,
                          dL, dH)

            def final_extra(b):
                nb = nb_of(b)
                hbT = wk.tile([P, 384], BF16, tag="hbT")
                for k, hsrc in enumerate((h2a, h2b, h2c)):
                    pt = ptrp.tile([P, 128], BF16, tag="pt")
                    nc.tensor.transpose(
                        out=pt[:, :nb], in_=hsrc(b, nb),
                        identity=identb[:nb, :nb])
                    nc.scalar.activation(out=hbT[:, k * 128:k * 128 + nb],
                                         in_=pt[:, :nb], func=AF.Copy)
                po = pm.tile([P, 256], F32, tag="pmm")
                for k in range(3):
                    nc.tensor.matmul(out=po[:nb, :128],
                                     lhsT=hbT[:, k * 128:k * 128 + nb],
                                     rhs=fpw_sb[k][:], start=(k == 0),
                                     stop=(k == 2))
                osb = wk.tile([P, 128], F32, tag="osb")
                nc.vector.tensor_tensor(out=osb[:nb, :], in0=po[:nb, :128],
                                        in1=bcast["fpb"][:nb, :], op=AL.add)
                nc.sync.dma_start(out=out_ext[b * BLK:b * BLK + nb, :],
                                  in_=osb[:nb, :])

            if _on('n4'):
                dL, dH = mk_second_app(h2c, LAM, extra=final_extra)
                spmm_pass("n4", tpn_full, nidx, nlrow, nval, sched_n, 128,
                          dL, dH)
            if STOP_AFTER is not None:
                dz = wk.tile([P, 128], F32, tag="osb")
                nc.vector.memset(dz[:], 0.0)
                for b in range(NB):
                    nb = nb_of(b)
                    nc.sync.dma_start(out=out_ext[b * BLK:b * BLK + nb, :],
                                      in_=dz[:nb, :])

    return nc


def _make(x, val, nd_val,
          l1_W0, l1_b0, l1_W1, l1_b1, l1_W2, l1_b2,
          l2_W0, l2_b0, l2_W1, l2_b1, l2_W2, l2_b2,
          bn_gamma, bn_beta, fp_W, fp_b,
          row, col, nd_row, nd_col):
    import ml_dtypes
    x = np.asarray(x, np.float32)
    row = np.asarray(row, np.int64); col = np.asarray(col, np.int64)
    val = np.asarray(val, np.float32)
    nd_row = np.asarray(nd_row, np.int64); nd_col = np.asarray(nd_col, np.int64)
    nd_val = np.asarray(nd_val, np.float32)

    a_i, a_l, a_v, sched_a, TA = _prep_adj(row, col, val)
    n_i, n_l, n_v, sched_n, TN = _prep_adj(nd_row, nd_col, nd_val)

    bf = ml_dtypes.bfloat16
    xt = np.ascontiguousarray(x.T).astype(bf)                # [128, N]
    iota = np.tile(np.arange(P, dtype=np.float32)[None, :], (P, 1))
    ident = np.eye(P, dtype=np.float32).astype(bf)
    ones1 = np.ones((1, P), np.float32)
    onesb = np.ones((P, 1), bf)
    w1h = np.ascontiguousarray(
        np.concatenate([l1_W1, l1_W2], 0).T).astype(bf)      # [128, 256]
    w10 = np.ascontiguousarray(
        (1.0 + LAM) * np.asarray(l1_W0, np.float32).T).astype(bf)
    w2h = np.ascontiguousarray(
        np.concatenate([l2_W1, l2_W2], 0).T).astype(bf)      # [384, 256]
    w20 = np.ascontiguousarray(
        (1.0 + LAM) * np.asarray(l2_W0, np.float32).T).astype(bf)
    fpw = np.ascontiguousarray(np.asarray(fp_W, np.float32).T).astype(bf)
    b2h = np.concatenate([l2_b1, l2_b2])[None, :].astype(np.float32)
    b20 = ((1.0 + LAM) * np.asarray(l2_b0))[None, :].astype(np.float32)
    fpb = np.asarray(fp_b)[None, :].astype(np.float32)
    bng = np.asarray(bn_gamma)[None, :].astype(np.float32)
    bnb = np.asarray(bn_beta)[None, :].astype(np.float32)

    nc = _build(TA, TN, sched_a, sched_n)
    # raw Bass skips Bacc's extended-inst codegen pass; without it the NEFF
    # compiler sees empty .instr on ISA subclasses -> "ISA wrong length"
    mybir.codegen_inst_isa_subclasses(nc)
    if FIX_WAITS:
        fix_waits(nc)

    in_maps = []
    for c in range(NCORES):
        in_maps.append({
            "xl": np.ascontiguousarray(xt[:, c * R:(c + 1) * R]),
            "a_idx": a_i[c], "a_lrow": a_l[c], "a_val": a_v[c],
            "n_idx": n_i[c], "n_lrow": n_l[c], "n_val": n_v[c],
            "iota": iota, "ident": ident, "ones1": ones1, "onesb": onesb,
            "w1h": w1h, "w10": w10, "w2h": w2h, "w20": w20, "fpw": fpw,
            "b2h": b2h, "b20": b20, "fpb": fpb, "bng": bng, "bnb": bnb,
        })
    return nc, in_maps


def kernel(**inputs):
    nc, in_maps = _make(**inputs)
    res = run_bass_kernel_spmd(nc, in_maps, list(range(NCORES)), trace=TRACE)
    LAST_RESULT["res"] = res
    out = np.concatenate([res.results[c]["out"] for c in range(NCORES)], axis=0)
    return out



# revision 3
# speedup vs baseline: 1.8008x; 1.8008x over previous
"""MixHop GNN (2 layers, 2 adjacencies, hops 0..2) on 8 trn2 NeuronCores.

Sharding: nodes row-partitioned across 8 cores (6250 rows each). Each SpMM
is computed for the core's destination rows only, gathering source rows
from a replicated DRAM table (AllGather between phases). The SpMM maps to
TensorE as a segment matmul: gather 128-edge tiles of source rows and
accumulate M^T @ G into a PSUM block of 128 destination rows, where
M[e, d] = val[e] * (lrow[e] == d).

v3 changes vs v2:
  * dma_gather descriptor prep (the v2 bottleneck: ~7.4ns/descriptor on Q7
    cores 0-1) is spread across all four SWDGE queues -- queue q's prep runs
    on Q7 cores 2q/2q+1, so round-robin queue assignment runs 4 preps
    concurrently (HW-measured 3.35x).
  * The val-scaled one-hot M matrices are precomputed on the host and
    streamed from DRAM (HWDGE, sequential 2KB/partition windows) instead of
    being built per-tile on VectorE (measured ~820ns/tile fixed-overhead
    floor -> 5.6ms total). A-passes share one M image; ND first/second
    applications use unscaled/lambda-scaled images so second-hop drains are
    plain adds.
  * BN-stats squaring moved to ScalarE (Square activation).

Self-contained: only numpy + ml_dtypes + concourse (environment packages).
"""
import numpy as np

import concourse.bass as bass
from concourse import mybir
from concourse.bass_utils import run_bass_kernel_spmd
from concourse.library_config import mlp
from concourse.tile import TileContext

F32 = mybir.dt.float32
BF16 = mybir.dt.bfloat16
I16 = mybir.dt.int16
AL = mybir.AluOpType
AF = mybir.ActivationFunctionType

N = 50000
NCORES = 8
R = N // NCORES          # 6250 rows per core
BLK = 128
NB = (R + BLK - 1) // BLK  # 49 blocks (48 full + 106)
LAM = 0.5
EPS = 1e-5
P = 128
HALF = 32768             # int16 gather-index limit
WIN = 8                  # 128-edge tiles per dma_gather call (1024 idxs =
                         # the SWDGE descriptor-ring per-call capacity)
NQ = 4                   # SWDGE queues; queue q's descriptor prep runs on Q7
                         # cores 2q/2q+1, so round-robin across 4 queues runs
                         # four preps concurrently instead of serializing on
                         # cores 0-1

TRACE = False            # set by test harness for profiling runs
FIX_WAITS = True         # disable for CoreSim validation (sim rejects
                         # post-hoc sync_info edits)
STOP_AFTER = None        # debug: truncate pipeline after a named stage
_STAGES = ['y12', 'hop0', 'a1', 'n1', 'a2', 'n2', 'bn', 'dense',
           'a3', 'n3', 'a4', 'n4']


def _on(stage):
    if STOP_AFTER is None:
        return True
    return _STAGES.index(stage) <= _STAGES.index(STOP_AFTER)
LAST_RESULT = {}


# ---------------------------------------------------------------- BIR post-pass
ASYNC_OPCODES = {"DMACopy", "CollectiveCompute", "DMAGatherAnt",
                 "DMAScatterAddAnt", "DMATransposeAnt"}


def _cap(inst) -> int:
    if inst.opcode in ("EventSemaphore", "NoOp"):
        return 999
    return 1


def fix_waits(nc, verbose=False):
    # --- collect streams (blocks concatenated in listed order; Tile output
    # is straight-line per engine)
    all_bbs = [bb for fn in nc.m.functions for bb in fn.blocks]
    streams = {}
    for bb in all_bbs:
        for inst in bb.instructions:
            streams.setdefault(inst.engine, []).append(inst)

    unsafe = set()
    wait_list = {}
    upd_list = {}
    for eng, insts in streams.items():
        for inst in insts:
            si = inst.sync_info
            ws, us = [], []
            if si:
                for w in (si.on_wait or []):
                    if getattr(w, "wait_mode", None) == "sem-ge-imm" and isinstance(
                            getattr(w, "wait_value", None), int):
                        ws.append((w.id, w.wait_value, w))
                    else:
                        ws.append((w.id, None, w))
                        unsafe.add(w.id)
                for u in (si.on_update or []):
                    um = getattr(u, "update_mode", None)
                    uv = getattr(u, "update_value", None)
                    if um == "sem-add-imm" and isinstance(uv, int):
                        us.append((u.id, uv))
                    elif um == "sem-inc":
                        us.append((u.id, 1))
                    else:
                        us.append((u.id, 0))
                        unsafe.add(u.id)
            wait_list[id(inst)] = ws
            upd_list[id(inst)] = us

    engines = list(streams.keys())
    ptr = {e: 0 for e in engines}
    vc = {e: {} for e in engines}
    sem_level = {}
    sem_cums = {}
    sem_snaps = {}

    def knowledge(s, v):
        cums = sem_cums.get(s)
        if not cums:
            return None
        import bisect
        i = bisect.bisect_left(cums, v)
        if i >= len(cums):
            i = len(cums) - 1
        return sem_snaps[s][i]

    n_dropped = 0
    progressed = True
    while progressed:
        progressed = False
        for eng in engines:
            insts = streams[eng]
            while ptr[eng] < len(insts):
                inst = insts[ptr[eng]]
                ws = wait_list[id(inst)]
                ok = True
                for (s, v, w) in ws:
                    if s in unsafe or v is None:
                        continue
                    if sem_level.get(s, 0) < v:
                        ok = False
                        break
                if not ok:
                    break
                myvc = vc[eng]
                kept = []
                # engine sems first: their knowledge usually implies the
                # DMA-lane waits, letting us drop the latter
                ws = sorted(ws, key=lambda t: str(
                    getattr(t[2], "ant_name", "")).startswith("DMA"))
                for (s, v, w) in ws:
                    if s not in unsafe and v is not None and myvc.get(s, 0) >= v:
                        n_dropped += 1
                        continue
                    kept.append(w)
                    if s in unsafe or v is None:
                        continue
                    k = knowledge(s, v)
                    if k:
                        for ks, kv in k.items():
                            if myvc.get(ks, 0) < kv:
                                myvc[ks] = kv
                    if myvc.get(s, 0) < v:
                        myvc[s] = v
                si = inst.sync_info
                if si and len(kept) != len(si.on_wait or []):
                    inst.sync_info = mybir.SyncInfo(
                        on_wait=kept, on_update=list(si.on_update or []))
                us = upd_list[id(inst)]
                if us:
                    is_async = inst.opcode in ASYNC_OPCODES
                    for (s, u) in us:
                        lvl = sem_level.get(s, 0) + u
                        sem_level[s] = lvl
                        if s not in unsafe:
                            snap = dict(myvc)
                            snap[s] = lvl
                            cums = sem_cums.setdefault(s, [])
                            snaps = sem_snaps.setdefault(s, [])
                            if snaps:
                                prev = snaps[-1]
                                for ks, kv in prev.items():
                                    if snap.get(ks, 0) < kv:
                                        snap[ks] = kv
                            cums.append(lvl)
                            snaps.append(snap)
                            if not is_async:
                                myvc[s] = lvl
                ptr[eng] += 1
                progressed = True

    stuck = sum(len(streams[e]) - ptr[e] for e in engines)
    # --- cap remaining waits with carriers
    uid = 0
    n_carriers = 0
    for bb in all_bbs:
        new_insts = []
        for inst in bb.instructions:
            si = inst.sync_info
            waits = list(si.on_wait) if (si and si.on_wait) else []
            cap = _cap(inst)
            if len(waits) > cap:
                keep = waits[len(waits) - cap:]
                for w in waits[: len(waits) - cap]:
                    uid += 1
                    new_insts.append(mybir.InstEventSemaphore(
                        name=f"waitfix-{uid}",
                        engine=inst.engine, ins=[], outs=[],
                        sync_info=mybir.SyncInfo(on_wait=[w], on_update=[]),
                    ))
                    n_carriers += 1
                inst.sync_info = mybir.SyncInfo(
                    on_wait=keep, on_update=list(si.on_update or []))
            new_insts.append(inst)
        bb.instructions = new_insts
    if verbose:
        print(f"fix_waits: dropped {n_dropped} redundant waits, "
              f"{n_carriers} carriers, {stuck} unprocessed")
    return nc


# ---------------------------------------------------------------- preprocessing
def _prep_adj(row, col, val):
    """Partition edges by destination core / 128-row block, then split each
    block's edges by source half (col < HALF vs >=) for int16 gather
    indices. Tile layout per pass: [all blocks' L tiles][all blocks' H
    tiles]; gather calls cover WIN-tile windows of each region.

    Returns:
      idx16 [NCORES, 128, T*8] int16  gather indices (wrapped 16-partition
                                      layout, replicated 8x)
      lrow_flat [NCORES, T*128]       local dest row per edge slot
      val_flat  [NCORES, T*128] f32   edge weight per slot (0 = pad)
      sched (TL, TH, [(b, lt0, kL, ht0, kH), ...])
      T = TL + TH total 128-edge tiles
    """
    row = np.asarray(row); col = np.asarray(col); val = np.asarray(val)
    core = row // R
    rloc = row - core * R
    blk = rloc // BLK
    lrow = rloc - blk * BLK
    half = (col >= HALF).astype(np.int64)
    idxval = np.where(half == 1, col - HALF, col).astype(np.int64)

    cnt = np.zeros((NCORES, NB, 2), np.int64)
    np.add.at(cnt, (core, blk, half), 1)
    kL = np.maximum(1, -(-cnt[:, :, 0].max(axis=0) // BLK))  # [NB]
    kH = np.maximum(1, -(-cnt[:, :, 1].max(axis=0) // BLK))  # [NB]

    lt0 = np.concatenate([[0], np.cumsum(kL)])
    TL = int(lt0[-1])
    ht0 = TL + np.concatenate([[0], np.cumsum(kH)])
    T = int(ht0[-1])
    sched = (TL, T - TL,
             [(b, int(lt0[b]), int(kL[b]), int(ht0[b]), int(kH[b]))
              for b in range(NB)])

    idx_flat = np.zeros((NCORES, T * BLK), np.int64)
    lrow_flat = np.zeros((NCORES, T * BLK), np.int64)
    val_flat = np.zeros((NCORES, T * BLK), np.float32)

    order = np.lexsort((col, half, blk, core))
    core_s, blk_s, half_s = core[order], blk[order], half[order]
    idx_s, lrow_s, val_s = idxval[order], lrow[order], val[order]
    key = (core_s * NB + blk_s) * 2 + half_s
    grid = np.arange(NCORES * NB * 2)
    starts = np.searchsorted(key, grid)
    ends = np.searchsorted(key, grid + 1)
    for c in range(NCORES):
        for b in range(NB):
            for h, base in ((0, lt0[b]), (1, ht0[b])):
                g = (c * NB + b) * 2 + h
                s, e = starts[g], ends[g]
                n = e - s
                if n == 0:
                    continue
                off = int(base) * BLK
                idx_flat[c, off:off + n] = idx_s[s:e]
                lrow_flat[c, off:off + n] = lrow_s[s:e]
                val_flat[c, off:off + n] = val_s[s:e]

    # idx wrap: call-local position i = t*128 + p lives at partition i%16,
    # column i//16 = t*8 + p//16 (call starts are tile-aligned, 128%16==0).
    arr = idx_flat.reshape(NCORES, T, 8, 16)           # [c, t, p//16, p%16]
    idx16 = arr.transpose(0, 3, 1, 2).reshape(NCORES, 16, T * 8)
    idx16 = np.ascontiguousarray(
        np.tile(idx16, (1, 8, 1)).astype(np.int16))    # [c, 128, T*8]
    return idx16, lrow_flat, val_flat, sched, T


def _mhost(lrow_flat, val_flat, T, scale=1.0):
    """Host-built M images: M[c][p, t*128 + j] = scale*val of the edge at
    (tile t, slot p) if lrow == j else 0.  [NCORES, 128, T*128] bf16."""
    import ml_dtypes
    M = np.zeros((NCORES, P, T * P), ml_dtypes.bfloat16)
    pos = np.arange(T * BLK)
    t = pos // BLK
    p = pos % BLK
    col = t * P + lrow_flat                            # [NCORES, T*128]
    v = (scale * val_flat).astype(ml_dtypes.bfloat16)
    for c in range(NCORES):
        M[c, p, col[c]] = v[c]
    return M


# ---------------------------------------------------------------- device program
def _build(TA, TN, sched_a, sched_n):
    nc = bass.Bass(num_devices=NCORES, num_swdge_queues=NQ)
    groups = [list(range(NCORES))]

    # ---- external I/O
    xl_in = nc.declare_dram_parameter("xl", [P, R], BF16, isOutput=False)
    a_idx = nc.declare_dram_parameter("a_idx", [P, TA * 8], I16, isOutput=False)
    n_idx = nc.declare_dram_parameter("n_idx", [P, TN * 8], I16, isOutput=False)
    m_a = nc.declare_dram_parameter("m_a", [P, TA * P], BF16, isOutput=False)
    m_nd = nc.declare_dram_parameter("m_nd", [P, TN * P], BF16, isOutput=False)
    m_ndl = nc.declare_dram_parameter("m_ndl", [P, TN * P], BF16, isOutput=False)
    ident_in = nc.declare_dram_parameter("ident", [P, P], BF16, isOutput=False)
    ones1_in = nc.declare_dram_parameter("ones1", [1, P], F32, isOutput=False)
    onesb_in = nc.declare_dram_parameter("onesb", [P, 1], BF16, isOutput=False)
    w1h_in = nc.declare_dram_parameter("w1h", [128, 256], BF16, isOutput=False)
    w10_in = nc.declare_dram_parameter("w10", [128, 128], BF16, isOutput=False)
    w2h_in = nc.declare_dram_parameter("w2h", [384, 256], BF16, isOutput=False)
    w20_in = nc.declare_dram_parameter("w20", [384, 128], BF16, isOutput=False)
    fpw_in = nc.declare_dram_parameter("fpw", [384, 128], BF16, isOutput=False)
    b2h_in = nc.declare_dram_parameter("b2h", [1, 256], F32, isOutput=False)
    b20_in = nc.declare_dram_parameter("b20", [1, 128], F32, isOutput=False)
    fpb_in = nc.declare_dram_parameter("fpb", [1, 128], F32, isOutput=False)
    bng_in = nc.declare_dram_parameter("bng", [1, 384], F32, isOutput=False)
    bnb_in = nc.declare_dram_parameter("bnb", [1, 384], F32, isOutput=False)
    out_ext = nc.declare_dram_parameter("out", [R, 128], F32, isOutput=True)

    with TileContext(nc) as tc:
        with (
            tc.tile_pool(name="consts", bufs=1) as consts,
            tc.tile_pool(name="glp", bufs=4) as glp,
            tc.tile_pool(name="mwp", bufs=4) as mwp,
            tc.tile_pool(name="wk", bufs=4) as wk,
            tc.tile_pool(name="ps", bufs=3, space="PSUM") as ps,
            tc.tile_pool(name="pm", bufs=1, space="PSUM") as pm,
            tc.tile_pool(name="ptr", bufs=2, space="PSUM") as ptrp,
            tc.tile_pool(name="pst", bufs=1, space="PSUM") as pstp,
            tc.tile_pool(name="dram", bufs=1, space="DRAM") as dram,
        ):
            nc.gpsimd.load_library(mlp)

            # num_idxs register cache: Pool has ~46 free registers and
            # to_reg() does not dedupe constants
            _regs = {}

            def nreg(v):
                if v not in _regs:
                    _regs[v] = nc.gpsimd.to_reg(v)
                return _regs[v]

            _qc = [0]

            def next_q():
                q = _qc[0]
                _qc[0] = (q + 1) % NQ
                return q

            # ---------------- DRAM scratch
            y12_loc = dram.tile([R, 256], BF16)
            ta_loc = dram.tile([R, 128], BF16)
            tn_loc = dram.tile([R, 128], BF16)
            z12_loc = dram.tile([R, 256], BF16)
            tpa_loc = dram.tile([R, 128], BF16)
            tpn_loc = dram.tile([R, 128], BF16)
            bn_loc = dram.tile([1, 768], F32)
            ss_dram = dram.tile([6, 128], F32)
            y12_full = dram.tile([N, 256], BF16, addr_space="Shared")
            ta_full = dram.tile([N, 128], BF16, addr_space="Shared")
            tn_full = dram.tile([N, 128], BF16, addr_space="Shared")
            z12_full = dram.tile([N, 256], BF16, addr_space="Shared")
            tpa_full = dram.tile([N, 128], BF16, addr_space="Shared")
            tpn_full = dram.tile([N, 128], BF16, addr_space="Shared")
            bn_full = dram.tile([1, 768], F32, addr_space="Shared")

            # ---------------- constants to SBUF
            def cload(src, shape, dtype):
                t = consts.tile(shape, dtype, name=f"c_{src.name}")
                nc.sync.dma_start(out=t[:], in_=src[:])
                return t

            xl = cload(xl_in, [P, R], BF16)
            identb = cload(ident_in, [P, P], BF16)
            ones1 = cload(ones1_in, [1, P], F32)
            onesb = cload(onesb_in, [P, 1], BF16)
            w1h = cload(w1h_in, [128, 256], BF16)
            w10 = cload(w10_in, [128, 128], BF16)
            b2h = cload(b2h_in, [1, 256], F32)
            b20 = cload(b20_in, [1, 128], F32)
            fpb = cload(fpb_in, [1, 128], F32)
            bng = cload(bng_in, [1, 384], F32)
            bnb = cload(bnb_in, [1, 384], F32)
            w2h_sb, w20_sb, fpw_sb = [], [], []
            for k in range(3):
                t = consts.tile([128, 256], BF16, name=f"w2h{k}")
                nc.sync.dma_start(out=t[:], in_=w2h_in[k * 128:(k + 1) * 128, :])
                w2h_sb.append(t)
                t = consts.tile([128, 128], BF16, name=f"w20{k}")
                nc.sync.dma_start(out=t[:], in_=w20_in[k * 128:(k + 1) * 128, :])
                w20_sb.append(t)
                t = consts.tile([128, 128], BF16, name=f"fpw{k}")
                nc.sync.dma_start(out=t[:], in_=fpw_in[k * 128:(k + 1) * 128, :])
                fpw_sb.append(t)

            aidx = cload(a_idx, [P, TA * 8], I16)
            nidx = cload(n_idx, [P, TN * 8], I16)

            # persistent per-node-block SBUF arrays. Layer-2's three hop
            # slices reuse h1_all's storage: block b's layer-1 features die
            # exactly when its layer-2 values are produced (hop0' is written
            # after the stage-7 transposes read the block; hop1'/hop2' are
            # written in later passes).
            h1_all = consts.tile([P, NB * 384], BF16, name="h1_all")
            sclT = consts.tile([P, 3], F32, name="sclT")
            shfT = consts.tile([P, 3], F32, name="shfT")

            def h2a(b, nb=P):
                return h1_all[:nb, b * 384:b * 384 + 128]

            def h2b(b, nb=P):
                return h1_all[:nb, b * 384 + 128:b * 384 + 256]

            def h2c(b, nb=P):
                return h1_all[:nb, b * 384 + 256:b * 384 + 384]

            def nb_of(b):
                return BLK if b < NB - 1 else R - BLK * (NB - 1)

            # broadcast bias rows to all partitions (b2h, b20*1.5, fpb)
            bcast = {}
            for nm, bsrc, wdt in (("b2h", b2h, 256), ("b20", b20, 128),
                                  ("fpb", fpb, 128)):
                pbx = pm.tile([P, 256], F32, tag="pmm", name=f"pb_{nm}")
                nc.tensor.matmul(out=pbx[:, :wdt], lhsT=ones1[:], rhs=bsrc[:],
                                 start=True, stop=True)
                bt = consts.tile([P, wdt], F32, name=f"bb_{nm}")
                nc.vector.tensor_copy(out=bt[:], in_=pbx[:, :wdt])
                bcast[nm] = bt

            # ---------------- stage 1: local Y12 = X @ [W1;W2]^T (no bias:
            # layer-1 biases cancel in BatchNorm), then AllGather.
            for b in range(NB) if _on('y12') else []:
                nb = nb_of(b)
                py = pm.tile([P, 256], F32, tag="pmm")
                nc.tensor.matmul(out=py[:nb, :], lhsT=xl[:, b * BLK:b * BLK + nb],
                                 rhs=w1h[:], start=True, stop=True)
                yb = wk.tile([P, 256], BF16, tag="yb")
                nc.scalar.activation(out=yb[:nb, :], in_=py[:nb, :], func=AF.Copy)
                nc.sync.dma_start(out=y12_loc[b * BLK:b * BLK + nb, :],
                                  in_=yb[:nb, :])
            if _on('y12'):
                nc.gpsimd.collective_compute(
                    "AllGather", AL.bypass, replica_groups=groups,
                    ins=[y12_loc[:]], outs=[y12_full[:]])

            # hop0 while the AllGather runs: h1 hop0 = 1.5 * x @ W0^T
            # (w10 pre-scaled by 1.5 on host; bias cancels in BN)
            for b in range(NB) if _on('hop0') else []:
                nb = nb_of(b)
                p0 = pm.tile([P, 256], F32, tag="pmm")
                nc.tensor.matmul(out=p0[:nb, :128], lhsT=xl[:, b * BLK:b * BLK + nb],
                                 rhs=w10[:], start=True, stop=True)
                nc.scalar.activation(out=h1_all[:nb, b * 384:b * 384 + 128],
                                     in_=p0[:nb, :128], func=AF.Copy)

            # ---------------- batched-gather SpMM pass: L phase over the
            # low table half, then H phase over the high half (int16 gather
            # indices). Gather calls cover WIN-tile windows, round-robin
            # across the 4 SWDGE queues; the matching M window streams in
            # via HWDGE. Each block accumulates one PSUM per phase; the H
            # drain combines with the L result.
            tstage = consts.tile([P, NB * 128], BF16, name="tstage")

            def spmm_phase(pfx, table_half, idxT, mdram, t0, ntiles,
                           binfo, elem, drain):
                wins = []
                mwins = []
                for wi, w0 in enumerate(range(0, ntiles, WIN)):
                    n = min(WIN, ntiles - w0)
                    g = glp.tile([P, WIN * elem], BF16, tag="gw",
                                 name=f"{pfx}g{w0}")
                    nc.gpsimd.dma_gather(
                        out_ap=g[:, :n * elem].rearrange(
                            "p (t e) -> p t e", e=elem),
                        in_ap=table_half,
                        idxs_ap=idxT[:, (t0 + w0) * 8:(t0 + w0 + n) * 8],
                        num_idxs=n * BLK, num_idxs_reg=nreg(n * BLK),
                        elem_size=elem, queue_num=next_q())
                    wins.append(g)
                    mw = mwp.tile([P, WIN * P], BF16, tag="mw",
                                  name=f"{pfx}m{w0}")
                    eng = nc.sync if wi % 2 == 0 else nc.scalar
                    eng.dma_start(out=mw[:, :n * P],
                                  in_=mdram[:, (t0 + w0) * P:(t0 + w0 + n) * P])
                    mwins.append(mw)
                for (b, bt0, k) in binfo:
                    psum = ps.tile([P, elem], F32, tag="sp", name=f"{pfx}ps{b}")
                    for i in range(k):
                        t = bt0 + i
                        lt = t - t0
                        g = wins[lt // WIN]
                        mw = mwins[lt // WIN]
                        s = lt % WIN
                        nc.tensor.matmul(
                            out=psum[:], lhsT=mw[:, s * P:(s + 1) * P],
                            rhs=g[:, s * elem:(s + 1) * elem],
                            start=(i == 0), stop=(i == k - 1))
                    drain(b, psum)

            def spmm_pass(pfx, table, idxT, mdram, sched, elem,
                          drainL, drainH):
                TL, TH, blocks = sched
                spmm_phase(pfx + "L", table[0:HALF, :], idxT, mdram,
                           0, TL,
                           [(b, lt0, kL) for (b, lt0, kL, _, _) in blocks],
                           elem, drainL)
                spmm_phase(pfx + "H", table[HALF:N, :], idxT, mdram,
                           TL, TH,
                           [(b, ht0, kH) for (b, _, _, ht0, kH) in blocks],
                           elem, drainH)

            # ---------------- layer 1, first application (hop1 + T tables)
            def mk_first_app(hslice, lam_first, t_loc):
                # hslice(b) -> target AP for the hop-1 slice; lam_first:
                # None for the A pass (copy), LAM for the ND pass (l-add)
                def dL(b, p):
                    sl = hslice(b)
                    if lam_first is None:
                        nc.vector.tensor_copy(out=sl, in_=p[:, 0:128])
                    else:
                        nc.vector.scalar_tensor_tensor(
                            out=sl, in0=p[:, 0:128], scalar=lam_first, in1=sl,
                            op0=AL.mult, op1=AL.add)
                    nc.scalar.activation(out=tstage[:, b * 128:(b + 1) * 128],
                                         in_=p[:, 128:256], func=AF.Copy)

                def dH(b, p):
                    nb = nb_of(b)
                    sl = hslice(b)
                    if lam_first is None:
                        nc.vector.tensor_tensor(out=sl, in0=p[:, 0:128],
                                                in1=sl, op=AL.add)
                    else:
                        nc.vector.scalar_tensor_tensor(
                            out=sl, in0=p[:, 0:128], scalar=lam_first, in1=sl,
                            op0=AL.mult, op1=AL.add)
                    tsb = wk.tile([P, 128], BF16, tag="tsb")
                    nc.vector.tensor_tensor(
                        out=tsb[:], in0=p[:, 128:256],
                        in1=tstage[:, b * 128:(b + 1) * 128], op=AL.add)
                    nc.sync.dma_start(out=t_loc[b * BLK:b * BLK + nb, :],
                                      in_=tsb[:nb, :])
                return dL, dH

            def h1_hop1(b, nb=P):
                return h1_all[:nb, b * 384 + 128:b * 384 + 256]

            def h1_hop2(b, nb=P):
                return h1_all[:nb, b * 384 + 256:b * 384 + 384]

            if _on('a1'):
                dL, dH = mk_first_app(h1_hop1, None, ta_loc)
                spmm_pass("a1", y12_full, aidx, m_a, sched_a, 256, dL, dH)
                nc.gpsimd.collective_compute(
                    "AllGather", AL.bypass, replica_groups=groups,
                    ins=[ta_loc[:]], outs=[ta_full[:]])
            if _on('n1'):
                dL, dH = mk_first_app(h1_hop1, LAM, tn_loc)
                spmm_pass("n1", y12_full, nidx, m_nd, sched_n, 256, dL, dH)
                nc.gpsimd.collective_compute(
                    "AllGather", AL.bypass, replica_groups=groups,
                    ins=[tn_loc[:]], outs=[tn_full[:]])

            # ---------------- layer 1, second application (hop2) + BN stats
            # ND second applications use the lambda-prescaled M image, so
            # every drain is a plain copy/add.
            def mk_second_app(hslice, init, extra=None):
                def dL(b, p):
                    sl = hslice(b)
                    if init:
                        nc.vector.tensor_copy(out=sl, in_=p[:, 0:128])
                    else:
                        nc.vector.tensor_tensor(out=sl, in0=p[:, 0:128],
                                                in1=sl, op=AL.add)

                def dH(b, p):
                    sl = hslice(b)
                    nc.vector.tensor_tensor(out=sl, in0=p[:, 0:128],
                                            in1=sl, op=AL.add)
                    if extra is not None:
                        extra(b)
                return dL, dH

            if _on('a2'):
                dL, dH = mk_second_app(h1_hop2, True)
                spmm_pass("a2", ta_full, aidx, m_a, sched_a, 128, dL, dH)

            pst = pstp.tile([1, 384], F32, tag="pst", name="pst")
            psq = pstp.tile([1, 384], F32, tag="psq", name="psq")

            def stats_extra(b):
                nb = nb_of(b)
                hsl = h1_all[:, b * 384:b * 384 + 384]
                sq = wk.tile([P, 384], BF16, tag="sq")
                nc.scalar.activation(out=sq[:nb, :], in_=hsl[:nb],
                                     func=AF.Square)
                nc.tensor.matmul(out=pst[:], lhsT=onesb[:nb, :], rhs=hsl[:nb],
                                 start=(b == 0), stop=(b == NB - 1))
                nc.tensor.matmul(out=psq[:], lhsT=onesb[:nb, :], rhs=sq[:nb, :],
                                 start=(b == 0), stop=(b == NB - 1))

            if _on('n2'):
                dL, dH = mk_second_app(h1_hop2, False, extra=stats_extra)
                spmm_pass("n2", tn_full, nidx, m_ndl, sched_n, 128, dL, dH)

            # ---------------- BN finalize (allreduce + feature-major params)
            if _on('bn'):
                stats = wk.tile([1, 768], F32, tag="bnst", bufs=1)
                nc.vector.tensor_copy(out=stats[:, 0:384], in_=pst[:])
                nc.vector.tensor_copy(out=stats[:, 384:768], in_=psq[:])
                nc.sync.dma_start(out=bn_loc[:], in_=stats[:])
                nc.gpsimd.collective_compute(
                    "AllReduce", AL.add, replica_groups=groups,
                    ins=[bn_loc[:]], outs=[bn_full[:]])
                bnr = wk.tile([1, 768], F32, tag="bnr", bufs=1)
                nc.sync.dma_start(out=bnr[:], in_=bn_full[:])
                mean = wk.tile([1, 384], F32, tag="bn1", bufs=1)
                var = wk.tile([1, 384], F32, tag="bn2", bufs=1)
                tmp = wk.tile([1, 384], F32, tag="bn3", bufs=1)
                sshf = wk.tile([1, 768], F32, tag="bn4", bufs=1)
                nc.vector.tensor_scalar(out=mean[:], in0=bnr[:, 0:384],
                                        scalar1=1.0 / N, scalar2=None, op0=AL.mult)
                nc.vector.tensor_scalar(out=var[:], in0=bnr[:, 384:768],
                                        scalar1=1.0 / N, scalar2=None, op0=AL.mult)
                nc.vector.tensor_tensor(out=tmp[:], in0=mean[:], in1=mean[:], op=AL.mult)
                nc.vector.tensor_tensor(out=var[:], in0=var[:], in1=tmp[:], op=AL.subtract)
                nc.vector.tensor_scalar(out=var[:], in0=var[:], scalar1=EPS,
                                        scalar2=None, op0=AL.add)
                nc.scalar.sqrt(out=var[:], in_=var[:])
                nc.vector.reciprocal(out=var[:], in_=var[:])
                nc.vector.tensor_tensor(out=sshf[:, 0:384], in0=bng[:], in1=var[:],
                                        op=AL.mult)
                nc.vector.tensor_tensor(out=tmp[:], in0=mean[:], in1=sshf[:, 0:384],
                                        op=AL.mult)
                nc.vector.tensor_tensor(out=sshf[:, 384:768], in0=bnb[:], in1=tmp[:],
                                        op=AL.subtract)
                nc.sync.dma_start(out=ss_dram[:], in_=sshf[:])
                for k in range(3):
                    nc.sync.dma_start(out=sclT[:, k:k + 1], in_=ss_dram[k:k + 1, :])
                    nc.sync.dma_start(out=shfT[:, k:k + 1], in_=ss_dram[k + 3:k + 4, :])

            # ---------------- layer-2 dense: transpose, fused BN+relu (ACT),
            # linears; z12 local then AllGather
            for b in range(NB) if _on('dense') else []:
                nb = nb_of(b)
                hbT = wk.tile([P, 384], BF16, tag="hbT")
                for k in range(3):
                    pt = ptrp.tile([P, 128], BF16, tag="pt")
                    nc.tensor.transpose(
                        out=pt[:, :nb],
                        in_=h1_all[:nb, b * 384 + k * 128:b * 384 + (k + 1) * 128],
                        identity=identb[:nb, :nb])
                    nc.scalar.activation(
                        out=hbT[:, k * 128:k * 128 + nb], in_=pt[:, :nb],
                        func=AF.Relu, scale=sclT[:, k:k + 1], bias=shfT[:, k:k + 1])
                pz = pm.tile([P, 256], F32, tag="pmm")
                for k in range(3):
                    nc.tensor.matmul(out=pz[:nb, :],
                                     lhsT=hbT[:, k * 128:k * 128 + nb],
                                     rhs=w2h_sb[k][:], start=(k == 0), stop=(k == 2))
                zt = wk.tile([P, 256], BF16, tag="yb")
                nc.vector.tensor_tensor(out=zt[:nb, :], in0=pz[:nb, :],
                                        in1=bcast["b2h"][:nb, :], op=AL.add)
                nc.sync.dma_start(out=z12_loc[b * BLK:b * BLK + nb, :], in_=zt[:nb, :])
                p0 = pm.tile([P, 256], F32, tag="pmm")
                for k in range(3):
                    nc.tensor.matmul(out=p0[:nb, :128],
                                     lhsT=hbT[:, k * 128:k * 128 + nb],
                                     rhs=w20_sb[k][:], start=(k == 0), stop=(k == 2))
                nc.vector.tensor_tensor(out=h2a(b, nb),
                                        in0=p0[:nb, :128], in1=bcast["b20"][:nb, :],
                                        op=AL.add)
            if _on('dense'):
                nc.gpsimd.collective_compute(
                    "AllGather", AL.bypass, replica_groups=groups,
                    ins=[z12_loc[:]], outs=[z12_full[:]])

            # ---------------- layer 2, first + second applications
            if _on('a3'):
                dL, dH = mk_first_app(h2b, None, tpa_loc)
                spmm_pass("a3", z12_full, aidx, m_a, sched_a, 256, dL, dH)
                nc.gpsimd.collective_compute(
                    "AllGather", AL.bypass, replica_groups=groups,
                    ins=[tpa_loc[:]], outs=[tpa_full[:]])
            if _on('n3'):
                dL, dH = mk_first_app(h2b, LAM, tpn_loc)
                spmm_pass("n3", z12_full, nidx, m_nd, sched_n, 256, dL, dH)
                nc.gpsimd.collective_compute(
                    "AllGather", AL.bypass, replica_groups=groups,
                    ins=[tpn_loc[:]], outs=[tpn_full[:]])

            if _on('a4'):
                dL, dH = mk_second_app(h2c, True)
                spmm_pass("a4", tpa_full, aidx, m_a, sched_a, 128, dL, dH)

            def final_extra(b):
                nb = nb_of(b)
                hbT = wk.tile([P, 384], BF16, tag="hbT")
                for k, hsrc in enumerate((h2a, h2b, h2c)):
                    pt = ptrp.tile([P, 128], BF16, tag="pt")
                    nc.tensor.transpose(
                        out=pt[:, :nb], in_=hsrc(b, nb),
                        identity=identb[:nb, :nb])
                    nc.scalar.activation(out=hbT[:, k * 128:k * 128 + nb],
                                         in_=pt[:, :nb], func=AF.Copy)
                po = pm.tile([P, 256], F32, tag="pmm")
                for k in range(3):
                    nc.tensor.matmul(out=po[:nb, :128],
                                     lhsT=hbT[:, k * 128:k * 128 + nb],
                                     rhs=fpw_sb[k][:], start=(k == 0),
                                     stop=(k == 2))
                osb = wk.tile([P, 128], F32, tag="osb")
                nc.vector.tensor_tensor(out=osb[:nb, :], in0=po[:nb, :128],
                                        in1=bcast["fpb"][:nb, :], op=AL.add)
                nc.sync.dma_start(out=out_ext[b * BLK:b * BLK + nb, :],
                                  in_=osb[:nb, :])

            if _on('n4'):
                dL, dH = mk_second_app(h2c, False, extra=final_extra)
                spmm_pass("n4", tpn_full, nidx, m_ndl, sched_n, 128, dL, dH)
            if STOP_AFTER is not None:
                dz = wk.tile([P, 128], F32, tag="osb")
                nc.vector.memset(dz[:], 0.0)
                for b in range(NB):
                    nb = nb_of(b)
                    nc.sync.dma_start(out=out_ext[b * BLK:b * BLK + nb, :],
                                      in_=dz[:nb, :])

    return nc


def _make(x, val, nd_val,
          l1_W0, l1_b0, l1_W1, l1_b1, l1_W2, l1_b2,
          l2_W0, l2_b0, l2_W1, l2_b1, l2_W2, l2_b2,
          bn_gamma, bn_beta, fp_W, fp_b,
          row, col, nd_row, nd_col):
    import ml_dtypes
    x = np.asarray(x, np.float32)
    row = np.asarray(row, np.int64); col = np.asarray(col, np.int64)
    val = np.asarray(val, np.float32)
    nd_row = np.asarray(nd_row, np.int64); nd_col = np.asarray(nd_col, np.int64)
    nd_val = np.asarray(nd_val, np.float32)

    a_i, a_lr, a_vf, sched_a, TA = _prep_adj(row, col, val)
    n_i, n_lr, n_vf, sched_n, TN = _prep_adj(nd_row, nd_col, nd_val)
    m_a = _mhost(a_lr, a_vf, TA)
    m_nd = _mhost(n_lr, n_vf, TN)
    m_ndl = _mhost(n_lr, n_vf, TN, scale=LAM)

    bf = ml_dtypes.bfloat16
    xt = np.ascontiguousarray(x.T).astype(bf)                # [128, N]
    ident = np.eye(P, dtype=np.float32).astype(bf)
    ones1 = np.ones((1, P), np.float32)
    onesb = np.ones((P, 1), bf)
    w1h = np.ascontiguousarray(
        np.concatenate([l1_W1, l1_W2], 0).T).astype(bf)      # [128, 256]
    w10 = np.ascontiguousarray(
        (1.0 + LAM) * np.asarray(l1_W0, np.float32).T).astype(bf)
    w2h = np.ascontiguousarray(
        np.concatenate([l2_W1, l2_W2], 0).T).astype(bf)      # [384, 256]
    w20 = np.ascontiguousarray(
        (1.0 + LAM) * np.asarray(l2_W0, np.float32).T).astype(bf)
    fpw = np.ascontiguousarray(np.asarray(fp_W, np.float32).T).astype(bf)
    b2h = np.concatenate([l2_b1, l2_b2])[None, :].astype(np.float32)
    b20 = ((1.0 + LAM) * np.asarray(l2_b0))[None, :].astype(np.float32)
    fpb = np.asarray(fp_b)[None, :].astype(np.float32)
    bng = np.asarray(bn_gamma)[None, :].astype(np.float32)
    bnb = np.asarray(bn_beta)[None, :].astype(np.float32)

    nc = _build(TA, TN, sched_a, sched_n)
    # raw Bass skips Bacc's extended-inst codegen pass; without it the NEFF
    # compiler sees empty .instr on ISA subclasses -> "ISA wrong length"
    mybir.codegen_inst_isa_subclasses(nc)
    if FIX_WAITS:
        fix_waits(nc)

    in_maps = []
    for c in range(NCORES):
        in_maps.append({
            "xl": np.ascontiguousarray(xt[:, c * R:(c + 1) * R]),
            "a_idx": a_i[c], "n_idx": n_i[c],
            "m_a": m_a[c], "m_nd": m_nd[c], "m_ndl": m_ndl[c],
            "ident": ident, "ones1": ones1, "onesb": onesb,
            "w1h": w1h, "w10": w10, "w2h": w2h, "w20": w20, "fpw": fpw,
            "b2h": b2h, "b20": b20, "fpb": fpb, "bng": bng, "bnb": bnb,
        })
    return nc, in_maps


def kernel(**inputs):
    nc, in_maps = _make(**inputs)
    res = run_bass_kernel_spmd(nc, in_maps, list(range(NCORES)), trace=TRACE)
    LAST_RESULT["res"] = res
    out = np.concatenate([res.results[c]["out"] for c in range(NCORES)], axis=0)
    return out


# revision 4
# speedup vs baseline: 1.8227x; 1.0121x over previous
"""MixHop GNN (2 layers, 2 adjacencies, hops 0..2) on 8 trn2 NeuronCores.

Sharding: nodes row-partitioned across 8 cores (6250 rows each). Each SpMM
is computed for the core's destination rows only, gathering source rows
from a replicated DRAM table (AllGather between phases). The SpMM maps to
TensorE as a segment matmul: gather 128-edge tiles of source rows and
accumulate M^T @ G into a PSUM block of 128 destination rows, where
M[e, d] = val[e] * (lrow[e] == d).

v3 changes vs v2:
  * dma_gather descriptor prep (the v2 bottleneck: ~7.4ns/descriptor on Q7
    cores 0-1) is spread across all four SWDGE queues -- queue q's prep runs
    on Q7 cores 2q/2q+1, so round-robin queue assignment runs 4 preps
    concurrently (HW-measured 3.35x).
  * The val-scaled one-hot M matrices are precomputed on the host and
    streamed from DRAM (HWDGE, sequential 2KB/partition windows) instead of
    being built per-tile on VectorE (measured ~820ns/tile fixed-overhead
    floor -> 5.6ms total). A-passes share one M image; ND first/second
    applications use unscaled/lambda-scaled images so second-hop drains are
    plain adds.
  * BN-stats squaring moved to ScalarE (Square activation).

Self-contained: only numpy + ml_dtypes + concourse (environment packages).
"""
import numpy as np

import concourse.bass as bass
from concourse import mybir
from concourse.bass_utils import run_bass_kernel_spmd
from concourse.library_config import mlp
from concourse.tile import TileContext

F32 = mybir.dt.float32
BF16 = mybir.dt.bfloat16
I16 = mybir.dt.int16
AL = mybir.AluOpType
AF = mybir.ActivationFunctionType

N = 50000
NCORES = 8
R = N // NCORES          # 6250 rows per core
BLK = 128
NB = (R + BLK - 1) // BLK  # 49 blocks (48 full + 106)
LAM = 0.5
EPS = 1e-5
P = 128
HALF = 32768             # int16 gather-index limit
WIN = 8                  # 128-edge tiles per dma_gather call (1024 idxs =
                         # the SWDGE descriptor-ring per-call capacity)
NQ = 4                   # SWDGE queues; queue q's descriptor prep runs on Q7
                         # cores 2q/2q+1, so round-robin across 4 queues runs
                         # four preps concurrently instead of serializing on
                         # cores 0-1

TRACE = False            # set by test harness for profiling runs
FIX_WAITS = True         # disable for CoreSim validation (sim rejects
                         # post-hoc sync_info edits)
STOP_AFTER = None        # debug: truncate pipeline after a named stage
_STAGES = ['y12', 'hop0', 'a1', 'n1', 'a2', 'n2', 'bn', 'dense',
           'a3', 'n3', 'a4', 'n4']


def _on(stage):
    if STOP_AFTER is None:
        return True
    return _STAGES.index(stage) <= _STAGES.index(STOP_AFTER)
LAST_RESULT = {}


# ---------------------------------------------------------------- BIR post-pass
ASYNC_OPCODES = {"DMACopy", "CollectiveCompute", "DMAGatherAnt",
                 "DMAScatterAddAnt", "DMATransposeAnt"}


def _cap(inst) -> int:
    if inst.opcode in ("EventSemaphore", "NoOp"):
        return 999
    return 1


def fix_waits(nc, verbose=False):
    # --- collect streams (blocks concatenated in listed order; Tile output
    # is straight-line per engine)
    all_bbs = [bb for fn in nc.m.functions for bb in fn.blocks]
    streams = {}
    for bb in all_bbs:
        for inst in bb.instructions:
            streams.setdefault(inst.engine, []).append(inst)

    unsafe = set()
    wait_list = {}
    upd_list = {}
    for eng, insts in streams.items():
        for inst in insts:
            si = inst.sync_info
            ws, us = [], []
            if si:
                for w in (si.on_wait or []):
                    if getattr(w, "wait_mode", None) == "sem-ge-imm" and isinstance(
                            getattr(w, "wait_value", None), int):
                        ws.append((w.id, w.wait_value, w))
                    else:
                        ws.append((w.id, None, w))
                        unsafe.add(w.id)
                for u in (si.on_update or []):
                    um = getattr(u, "update_mode", None)
                    uv = getattr(u, "update_value", None)
                    if um == "sem-add-imm" and isinstance(uv, int):
                        us.append((u.id, uv))
                    elif um == "sem-inc":
                        us.append((u.id, 1))
                    else:
                        us.append((u.id, 0))
                        unsafe.add(u.id)
            wait_list[id(inst)] = ws
            upd_list[id(inst)] = us

    engines = list(streams.keys())
    ptr = {e: 0 for e in engines}
    vc = {e: {} for e in engines}
    sem_level = {}
    sem_cums = {}
    sem_snaps = {}

    def knowledge(s, v):
        cums = sem_cums.get(s)
        if not cums:
            return None
        import bisect
        i = bisect.bisect_left(cums, v)
        if i >= len(cums):
            i = len(cums) - 1
        return sem_snaps[s][i]

    n_dropped = 0
    progressed = True
    while progressed:
        progressed = False
        for eng in engines:
            insts = streams[eng]
            while ptr[eng] < len(insts):
                inst = insts[ptr[eng]]
                ws = wait_list[id(inst)]
                ok = True
                for (s, v, w) in ws:
                    if s in unsafe or v is None:
                        continue
                    if sem_level.get(s, 0) < v:
                        ok = False
                        break
                if not ok:
                    break
                myvc = vc[eng]
                kept = []
                # engine sems first: their knowledge usually implies the
                # DMA-lane waits, letting us drop the latter
                ws = sorted(ws, key=lambda t: str(
                    getattr(t[2], "ant_name", "")).startswith("DMA"))
                for (s, v, w) in ws:
                    if s not in unsafe and v is not None and myvc.get(s, 0) >= v:
                        n_dropped += 1
                        continue
                    kept.append(w)
                    if s in unsafe or v is None:
                        continue
                    k = knowledge(s, v)
                    if k:
                        for ks, kv in k.items():
                            if myvc.get(ks, 0) < kv:
                                myvc[ks] = kv
                    if myvc.get(s, 0) < v:
                        myvc[s] = v
                si = inst.sync_info
                if si and len(kept) != len(si.on_wait or []):
                    inst.sync_info = mybir.SyncInfo(
                        on_wait=kept, on_update=list(si.on_update or []))
                us = upd_list[id(inst)]
                if us:
                    is_async = inst.opcode in ASYNC_OPCODES
                    for (s, u) in us:
                        lvl = sem_level.get(s, 0) + u
                        sem_level[s] = lvl
                        if s not in unsafe:
                            snap = dict(myvc)
                            snap[s] = lvl
                            cums = sem_cums.setdefault(s, [])
                            snaps = sem_snaps.setdefault(s, [])
                            if snaps:
                                prev = snaps[-1]
                                for ks, kv in prev.items():
                                    if snap.get(ks, 0) < kv:
                                        snap[ks] = kv
                            cums.append(lvl)
                            snaps.append(snap)
                            if not is_async:
                                myvc[s] = lvl
                ptr[eng] += 1
                progressed = True

    stuck = sum(len(streams[e]) - ptr[e] for e in engines)
    # --- cap remaining waits with carriers
    uid = 0
    n_carriers = 0
    for bb in all_bbs:
        new_insts = []
        for inst in bb.instructions:
            si = inst.sync_info
            waits = list(si.on_wait) if (si and si.on_wait) else []
            cap = _cap(inst)
            if len(waits) > cap:
                keep = waits[len(waits) - cap:]
                for w in waits[: len(waits) - cap]:
                    uid += 1
                    new_insts.append(mybir.InstEventSemaphore(
                        name=f"waitfix-{uid}",
                        engine=inst.engine, ins=[], outs=[],
                        sync_info=mybir.SyncInfo(on_wait=[w], on_update=[]),
                    ))
                    n_carriers += 1
                inst.sync_info = mybir.SyncInfo(
                    on_wait=keep, on_update=list(si.on_update or []))
            new_insts.append(inst)
        bb.instructions = new_insts
    if verbose:
        print(f"fix_waits: dropped {n_dropped} redundant waits, "
              f"{n_carriers} carriers, {stuck} unprocessed")
    return nc


# ---------------------------------------------------------------- preprocessing
def _prep_adj(row, col, val):
    """Partition edges by destination core / 128-row block, then split each
    block's edges by source half (col < HALF vs >=) for int16 gather
    indices. Tile layout per pass: [all blocks' L tiles][all blocks' H
    tiles]; gather calls cover WIN-tile windows of each region.

    Returns:
      idx16 [NCORES, 128, T*8] int16  gather indices (wrapped 16-partition
                                      layout, replicated 8x)
      lrow_flat [NCORES, T*128]       local dest row per edge slot
      val_flat  [NCORES, T*128] f32   edge weight per slot (0 = pad)
      sched (TL, TH, [(b, lt0, kL, ht0, kH), ...])
      T = TL + TH total 128-edge tiles
    """
    row = np.asarray(row); col = np.asarray(col); val = np.asarray(val)
    core = row // R
    rloc = row - core * R
    blk = rloc // BLK
    lrow = rloc - blk * BLK
    half = (col >= HALF).astype(np.int64)
    idxval = np.where(half == 1, col - HALF, col).astype(np.int64)

    cnt = np.zeros((NCORES, NB, 2), np.int64)
    np.add.at(cnt, (core, blk, half), 1)
    kL = np.maximum(1, -(-cnt[:, :, 0].max(axis=0) // BLK))  # [NB]
    kH = np.maximum(1, -(-cnt[:, :, 1].max(axis=0) // BLK))  # [NB]

    lt0 = np.concatenate([[0], np.cumsum(kL)])
    TL = int(lt0[-1])
    ht0 = TL + np.concatenate([[0], np.cumsum(kH)])
    T = int(ht0[-1])
    sched = (TL, T - TL,
             [(b, int(lt0[b]), int(kL[b]), int(ht0[b]), int(kH[b]))
              for b in range(NB)])

    idx_flat = np.zeros((NCORES, T * BLK), np.int64)
    lrow_flat = np.zeros((NCORES, T * BLK), np.int64)
    val_flat = np.zeros((NCORES, T * BLK), np.float32)

    order = np.lexsort((col, half, blk, core))
    core_s, blk_s, half_s = core[order], blk[order], half[order]
    idx_s, lrow_s, val_s = idxval[order], lrow[order], val[order]
    key = (core_s * NB + blk_s) * 2 + half_s
    grid = np.arange(NCORES * NB * 2)
    starts = np.searchsorted(key, grid)
    ends = np.searchsorted(key, grid + 1)
    for c in range(NCORES):
        for b in range(NB):
            for h, base in ((0, lt0[b]), (1, ht0[b])):
                g = (c * NB + b) * 2 + h
                s, e = starts[g], ends[g]
                n = e - s
                if n == 0:
                    continue
                off = int(base) * BLK
                idx_flat[c, off:off + n] = idx_s[s:e]
                lrow_flat[c, off:off + n] = lrow_s[s:e]
                val_flat[c, off:off + n] = val_s[s:e]

    # idx wrap: call-local position i = t*128 + p lives at partition i%16,
    # column i//16 = t*8 + p//16 (call starts are tile-aligned, 128%16==0).
    arr = idx_flat.reshape(NCORES, T, 8, 16)           # [c, t, p//16, p%16]
    idx16 = arr.transpose(0, 3, 1, 2).reshape(NCORES, 16, T * 8)
    idx16 = np.ascontiguousarray(
        np.tile(idx16, (1, 8, 1)).astype(np.int16))    # [c, 128, T*8]
    return idx16, lrow_flat, val_flat, sched, T


def _mhost(lrow_flat, val_flat, T, scale=1.0):
    """Host-built M images: M[c][p, t*128 + j] = scale*val of the edge at
    (tile t, slot p) if lrow == j else 0.  [NCORES, 128, T*128] bf16."""
    import ml_dtypes
    M = np.zeros((NCORES, P, T * P), ml_dtypes.bfloat16)
    pos = np.arange(T * BLK)
    t = pos // BLK
    p = pos % BLK
    col = t * P + lrow_flat                            # [NCORES, T*128]
    v = (scale * val_flat).astype(ml_dtypes.bfloat16)
    for c in range(NCORES):
        M[c, p, col[c]] = v[c]
    return M


# ---------------------------------------------------------------- device program
def _build(TA, TN, sched_a, sched_n):
    nc = bass.Bass(num_devices=NCORES, num_swdge_queues=NQ)
    groups = [list(range(NCORES))]

    # ---- external I/O
    xl_in = nc.declare_dram_parameter("xl", [P, R], BF16, isOutput=False)
    a_idx = nc.declare_dram_parameter("a_idx", [P, TA * 8], I16, isOutput=False)
    n_idx = nc.declare_dram_parameter("n_idx", [P, TN * 8], I16, isOutput=False)
    m_a = nc.declare_dram_parameter("m_a", [P, TA * P], BF16, isOutput=False)
    m_nd = nc.declare_dram_parameter("m_nd", [P, TN * P], BF16, isOutput=False)
    m_ndl = nc.declare_dram_parameter("m_ndl", [P, TN * P], BF16, isOutput=False)
    ident_in = nc.declare_dram_parameter("ident", [P, P], BF16, isOutput=False)
    ones1_in = nc.declare_dram_parameter("ones1", [1, P], F32, isOutput=False)
    onesb_in = nc.declare_dram_parameter("onesb", [P, 1], BF16, isOutput=False)
    w1h_in = nc.declare_dram_parameter("w1h", [128, 256], BF16, isOutput=False)
    w10_in = nc.declare_dram_parameter("w10", [128, 128], BF16, isOutput=False)
    w2h_in = nc.declare_dram_parameter("w2h", [384, 256], BF16, isOutput=False)
    w20_in = nc.declare_dram_parameter("w20", [384, 128], BF16, isOutput=False)
    fpw_in = nc.declare_dram_parameter("fpw", [384, 128], BF16, isOutput=False)
    b2h_in = nc.declare_dram_parameter("b2h", [1, 256], F32, isOutput=False)
    b20_in = nc.declare_dram_parameter("b20", [1, 128], F32, isOutput=False)
    fpb_in = nc.declare_dram_parameter("fpb", [1, 128], F32, isOutput=False)
    bng_in = nc.declare_dram_parameter("bng", [1, 384], F32, isOutput=False)
    bnb_in = nc.declare_dram_parameter("bnb", [1, 384], F32, isOutput=False)
    out_ext = nc.declare_dram_parameter("out", [R, 128], F32, isOutput=True)

    with TileContext(nc) as tc:
        with (
            tc.tile_pool(name="consts", bufs=1) as consts,
            tc.tile_pool(name="glp", bufs=8) as glp,
            tc.tile_pool(name="mwp", bufs=8) as mwp,
            tc.tile_pool(name="wk", bufs=4) as wk,
            tc.tile_pool(name="ps", bufs=3, space="PSUM") as ps,
            tc.tile_pool(name="pm", bufs=1, space="PSUM") as pm,
            tc.tile_pool(name="ptr", bufs=2, space="PSUM") as ptrp,
            tc.tile_pool(name="pst", bufs=1, space="PSUM") as pstp,
            tc.tile_pool(name="dram", bufs=1, space="DRAM") as dram,
        ):
            nc.gpsimd.load_library(mlp)

            # num_idxs register cache: Pool has ~46 free registers and
            # to_reg() does not dedupe constants
            _regs = {}

            def nreg(v):
                if v not in _regs:
                    _regs[v] = nc.gpsimd.to_reg(v)
                return _regs[v]

            _qc = [0]

            def next_q():
                q = _qc[0]
                _qc[0] = (q + 1) % NQ
                return q

            # ---------------- DRAM scratch
            y12_loc = dram.tile([R, 256], BF16)
            ta_loc = dram.tile([R, 128], BF16)
            tn_loc = dram.tile([R, 128], BF16)
            z12_loc = dram.tile([R, 256], BF16)
            tpa_loc = dram.tile([R, 128], BF16)
            tpn_loc = dram.tile([R, 128], BF16)
            bn_loc = dram.tile([1, 768], F32)
            ss_dram = dram.tile([6, 128], F32)
            y12_full = dram.tile([N, 256], BF16, addr_space="Shared")
            ta_full = dram.tile([N, 128], BF16, addr_space="Shared")
            tn_full = dram.tile([N, 128], BF16, addr_space="Shared")
            z12_full = dram.tile([N, 256], BF16, addr_space="Shared")
            tpa_full = dram.tile([N, 128], BF16, addr_space="Shared")
            tpn_full = dram.tile([N, 128], BF16, addr_space="Shared")
            bn_full = dram.tile([1, 768], F32, addr_space="Shared")

            # ---------------- constants to SBUF
            def cload(src, shape, dtype):
                t = consts.tile(shape, dtype, name=f"c_{src.name}")
                nc.sync.dma_start(out=t[:], in_=src[:])
                return t

            xl = cload(xl_in, [P, R], BF16)
            identb = cload(ident_in, [P, P], BF16)
            ones1 = cload(ones1_in, [1, P], F32)
            onesb = cload(onesb_in, [P, 1], BF16)
            w1h = cload(w1h_in, [128, 256], BF16)
            w10 = cload(w10_in, [128, 128], BF16)
            b2h = cload(b2h_in, [1, 256], F32)
            b20 = cload(b20_in, [1, 128], F32)
            fpb = cload(fpb_in, [1, 128], F32)
            bng = cload(bng_in, [1, 384], F32)
            bnb = cload(bnb_in, [1, 384], F32)
            w2h_sb, w20_sb, fpw_sb = [], [], []
            for k in range(3):
                t = consts.tile([128, 256], BF16, name=f"w2h{k}")
                nc.sync.dma_start(out=t[:], in_=w2h_in[k * 128:(k + 1) * 128, :])
                w2h_sb.append(t)
                t = consts.tile([128, 128], BF16, name=f"w20{k}")
                nc.sync.dma_start(out=t[:], in_=w20_in[k * 128:(k + 1) * 128, :])
                w20_sb.append(t)
                t = consts.tile([128, 128], BF16, name=f"fpw{k}")
                nc.sync.dma_start(out=t[:], in_=fpw_in[k * 128:(k + 1) * 128, :])
                fpw_sb.append(t)

            aidx = cload(a_idx, [P, TA * 8], I16)
            nidx = cload(n_idx, [P, TN * 8], I16)

            # persistent per-node-block SBUF arrays. Layer-2's three hop
            # slices reuse h1_all's storage: block b's layer-1 features die
            # exactly when its layer-2 values are produced (hop0' is written
            # after the stage-7 transposes read the block; hop1'/hop2' are
            # written in later passes).
            h1_all = consts.tile([P, NB * 384], BF16, name="h1_all")
            sclT = consts.tile([P, 3], F32, name="sclT")
            shfT = consts.tile([P, 3], F32, name="shfT")

            def h2a(b, nb=P):
                return h1_all[:nb, b * 384:b * 384 + 128]

            def h2b(b, nb=P):
                return h1_all[:nb, b * 384 + 128:b * 384 + 256]

            def h2c(b, nb=P):
                return h1_all[:nb, b * 384 + 256:b * 384 + 384]

            def nb_of(b):
                return BLK if b < NB - 1 else R - BLK * (NB - 1)

            # broadcast bias rows to all partitions (b2h, b20*1.5, fpb)
            bcast = {}
            for nm, bsrc, wdt in (("b2h", b2h, 256), ("b20", b20, 128),
                                  ("fpb", fpb, 128)):
                pbx = pm.tile([P, 256], F32, tag="pmm", name=f"pb_{nm}")
                nc.tensor.matmul(out=pbx[:, :wdt], lhsT=ones1[:], rhs=bsrc[:],
                                 start=True, stop=True)
                bt = consts.tile([P, wdt], F32, name=f"bb_{nm}")
                nc.vector.tensor_copy(out=bt[:], in_=pbx[:, :wdt])
                bcast[nm] = bt

            # ---------------- stage 1: local Y12 = X @ [W1;W2]^T (no bias:
            # layer-1 biases cancel in BatchNorm), then AllGather.
            for b in range(NB) if _on('y12') else []:
                nb = nb_of(b)
                py = pm.tile([P, 256], F32, tag="pmm")
                nc.tensor.matmul(out=py[:nb, :], lhsT=xl[:, b * BLK:b * BLK + nb],
                                 rhs=w1h[:], start=True, stop=True)
                yb = wk.tile([P, 256], BF16, tag="yb")
                nc.scalar.activation(out=yb[:nb, :], in_=py[:nb, :], func=AF.Copy)
                nc.sync.dma_start(out=y12_loc[b * BLK:b * BLK + nb, :],
                                  in_=yb[:nb, :])
            if _on('y12'):
                nc.gpsimd.collective_compute(
                    "AllGather", AL.bypass, replica_groups=groups,
                    ins=[y12_loc[:]], outs=[y12_full[:]])

            # hop0 while the AllGather runs: h1 hop0 = 1.5 * x @ W0^T
            # (w10 pre-scaled by 1.5 on host; bias cancels in BN)
            for b in range(NB) if _on('hop0') else []:
                nb = nb_of(b)
                p0 = pm.tile([P, 256], F32, tag="pmm")
                nc.tensor.matmul(out=p0[:nb, :128], lhsT=xl[:, b * BLK:b * BLK + nb],
                                 rhs=w10[:], start=True, stop=True)
                nc.scalar.activation(out=h1_all[:nb, b * 384:b * 384 + 128],
                                     in_=p0[:nb, :128], func=AF.Copy)

            # ---------------- batched-gather SpMM pass: L phase over the
            # low table half, then H phase over the high half (int16 gather
            # indices). Gather calls cover WIN-tile windows, round-robin
            # across the 4 SWDGE queues; the matching M window streams in
            # via HWDGE. Each block accumulates one PSUM per phase; the H
            # drain combines with the L result.
            tstage = consts.tile([P, NB * 128], BF16, name="tstage")

            def spmm_phase(pfx, table_half, idxT, mdram, t0, ntiles,
                           binfo, elem, drain):
                wins = []
                mwins = []
                for wi, w0 in enumerate(range(0, ntiles, WIN)):
                    n = min(WIN, ntiles - w0)
                    g = glp.tile([P, WIN * elem], BF16, tag="gw",
                                 name=f"{pfx}g{w0}")
                    nc.gpsimd.dma_gather(
                        out_ap=g[:, :n * elem].rearrange(
                            "p (t e) -> p t e", e=elem),
                        in_ap=table_half,
                        idxs_ap=idxT[:, (t0 + w0) * 8:(t0 + w0 + n) * 8],
                        num_idxs=n * BLK, num_idxs_reg=nreg(n * BLK),
                        elem_size=elem, queue_num=next_q())
                    wins.append(g)
                    mw = mwp.tile([P, WIN * P], BF16, tag="mw",
                                  name=f"{pfx}m{w0}")
                    eng = nc.sync if wi % 2 == 0 else nc.scalar
                    eng.dma_start(out=mw[:, :n * P],
                                  in_=mdram[:, (t0 + w0) * P:(t0 + w0 + n) * P])
                    mwins.append(mw)
                for (b, bt0, k) in binfo:
                    psum = ps.tile([P, elem], F32, tag="sp", name=f"{pfx}ps{b}")
                    for i in range(k):
                        t = bt0 + i
                        lt = t - t0
                        g = wins[lt // WIN]
                        mw = mwins[lt // WIN]
                        s = lt % WIN
                        nc.tensor.matmul(
                            out=psum[:], lhsT=mw[:, s * P:(s + 1) * P],
                            rhs=g[:, s * elem:(s + 1) * elem],
                            start=(i == 0), stop=(i == k - 1))
                    drain(b, psum)

            def spmm_pass(pfx, table, idxT, mdram, sched, elem,
                          drainL, drainH):
                TL, TH, blocks = sched
                spmm_phase(pfx + "L", table[0:HALF, :], idxT, mdram,
                           0, TL,
                           [(b, lt0, kL) for (b, lt0, kL, _, _) in blocks],
                           elem, drainL)
                spmm_phase(pfx + "H", table[HALF:N, :], idxT, mdram,
                           TL, TH,
                           [(b, ht0, kH) for (b, _, _, ht0, kH) in blocks],
                           elem, drainH)

            # ---------------- layer 1, first application (hop1 + T tables)
            def mk_first_app(hslice, lam_first, t_loc):
                # hslice(b) -> target AP for the hop-1 slice; lam_first:
                # None for the A pass (copy), LAM for the ND pass (l-add)
                def dL(b, p):
                    sl = hslice(b)
                    if lam_first is None:
                        nc.vector.tensor_copy(out=sl, in_=p[:, 0:128])
                    else:
                        nc.vector.scalar_tensor_tensor(
                            out=sl, in0=p[:, 0:128], scalar=lam_first, in1=sl,
                            op0=AL.mult, op1=AL.add)
                    nc.scalar.activation(out=tstage[:, b * 128:(b + 1) * 128],
                                         in_=p[:, 128:256], func=AF.Copy)

                def dH(b, p):
                    nb = nb_of(b)
                    sl = hslice(b)
                    if lam_first is None:
                        nc.vector.tensor_tensor(out=sl, in0=p[:, 0:128],
                                                in1=sl, op=AL.add)
                    else:
                        nc.vector.scalar_tensor_tensor(
                            out=sl, in0=p[:, 0:128], scalar=lam_first, in1=sl,
                            op0=AL.mult, op1=AL.add)
                    tsb = wk.tile([P, 128], BF16, tag="tsb")
                    nc.vector.tensor_tensor(
                        out=tsb[:], in0=p[:, 128:256],
                        in1=tstage[:, b * 128:(b + 1) * 128], op=AL.add)
                    nc.sync.dma_start(out=t_loc[b * BLK:b * BLK + nb, :],
                                      in_=tsb[:nb, :])
                return dL, dH

            def h1_hop1(b, nb=P):
                return h1_all[:nb, b * 384 + 128:b * 384 + 256]

            def h1_hop2(b, nb=P):
                return h1_all[:nb, b * 384 + 256:b * 384 + 384]

            if _on('a1'):
                dL, dH = mk_first_app(h1_hop1, None, ta_loc)
                spmm_pass("a1", y12_full, aidx, m_a, sched_a, 256, dL, dH)
                nc.gpsimd.collective_compute(
                    "AllGather", AL.bypass, replica_groups=groups,
                    ins=[ta_loc[:]], outs=[ta_full[:]])
            if _on('n1'):
                dL, dH = mk_first_app(h1_hop1, LAM, tn_loc)
                spmm_pass("n1", y12_full, nidx, m_nd, sched_n, 256, dL, dH)
                nc.gpsimd.collective_compute(
                    "AllGather", AL.bypass, replica_groups=groups,
                    ins=[tn_loc[:]], outs=[tn_full[:]])

            # ---------------- layer 1, second application (hop2) + BN stats
            # ND second applications use the lambda-prescaled M image, so
            # every drain is a plain copy/add.
            def mk_second_app(hslice, init, extra=None):
                def dL(b, p):
                    sl = hslice(b)
                    if init:
                        nc.vector.tensor_copy(out=sl, in_=p[:, 0:128])
                    else:
                        nc.vector.tensor_tensor(out=sl, in0=p[:, 0:128],
                                                in1=sl, op=AL.add)

                def dH(b, p):
                    sl = hslice(b)
                    nc.vector.tensor_tensor(out=sl, in0=p[:, 0:128],
                                            in1=sl, op=AL.add)
                    if extra is not None:
                        extra(b)
                return dL, dH

            if _on('a2'):
                dL, dH = mk_second_app(h1_hop2, True)
                spmm_pass("a2", ta_full, aidx, m_a, sched_a, 128, dL, dH)

            pst = pstp.tile([1, 384], F32, tag="pst", name="pst")
            psq = pstp.tile([1, 384], F32, tag="psq", name="psq")

            def stats_extra(b):
                nb = nb_of(b)
                hsl = h1_all[:, b * 384:b * 384 + 384]
                sq = wk.tile([P, 384], BF16, tag="sq")
                nc.scalar.activation(out=sq[:nb, :], in_=hsl[:nb],
                                     func=AF.Square)
                nc.tensor.matmul(out=pst[:], lhsT=onesb[:nb, :], rhs=hsl[:nb],
                                 start=(b == 0), stop=(b == NB - 1))
                nc.tensor.matmul(out=psq[:], lhsT=onesb[:nb, :], rhs=sq[:nb, :],
                                 start=(b == 0), stop=(b == NB - 1))

            if _on('n2'):
                dL, dH = mk_second_app(h1_hop2, False, extra=stats_extra)
                spmm_pass("n2", tn_full, nidx, m_ndl, sched_n, 128, dL, dH)

            # ---------------- BN finalize (allreduce + feature-major params)
            if _on('bn'):
                stats = wk.tile([1, 768], F32, tag="bnst", bufs=1)
                nc.vector.tensor_copy(out=stats[:, 0:384], in_=pst[:])
                nc.vector.tensor_copy(out=stats[:, 384:768], in_=psq[:])
                nc.sync.dma_start(out=bn_loc[:], in_=stats[:])
                nc.gpsimd.collective_compute(
                    "AllReduce", AL.add, replica_groups=groups,
                    ins=[bn_loc[:]], outs=[bn_full[:]])
                bnr = wk.tile([1, 768], F32, tag="bnr", bufs=1)
                nc.sync.dma_start(out=bnr[:], in_=bn_full[:])
                mean = wk.tile([1, 384], F32, tag="bn1", bufs=1)
                var = wk.tile([1, 384], F32, tag="bn2", bufs=1)
                tmp = wk.tile([1, 384], F32, tag="bn3", bufs=1)
                sshf = wk.tile([1, 768], F32, tag="bn4", bufs=1)
                nc.vector.tensor_scalar(out=mean[:], in0=bnr[:, 0:384],
                                        scalar1=1.0 / N, scalar2=None, op0=AL.mult)
                nc.vector.tensor_scalar(out=var[:], in0=bnr[:, 384:768],
                                        scalar1=1.0 / N, scalar2=None, op0=AL.mult)
                nc.vector.tensor_tensor(out=tmp[:], in0=mean[:], in1=mean[:], op=AL.mult)
                nc.vector.tensor_tensor(out=var[:], in0=var[:], in1=tmp[:], op=AL.subtract)
                nc.vector.tensor_scalar(out=var[:], in0=var[:], scalar1=EPS,
                                        scalar2=None, op0=AL.add)
                nc.scalar.sqrt(out=var[:], in_=var[:])
                nc.vector.reciprocal(out=var[:], in_=var[:])
                nc.vector.tensor_tensor(out=sshf[:, 0:384], in0=bng[:], in1=var[:],
                                        op=AL.mult)
                nc.vector.tensor_tensor(out=tmp[:], in0=mean[:], in1=sshf[:, 0:384],
                                        op=AL.mult)
                nc.vector.tensor_tensor(out=sshf[:, 384:768], in0=bnb[:], in1=tmp[:],
                                        op=AL.subtract)
                nc.sync.dma_start(out=ss_dram[:], in_=sshf[:])
                for k in range(3):
                    nc.sync.dma_start(out=sclT[:, k:k + 1], in_=ss_dram[k:k + 1, :])
                    nc.sync.dma_start(out=shfT[:, k:k + 1], in_=ss_dram[k + 3:k + 4, :])

            # ---------------- layer-2 dense: transpose, fused BN+relu (ACT),
            # linears; z12 local then AllGather
            for b in range(NB) if _on('dense') else []:
                nb = nb_of(b)
                hbT = wk.tile([P, 384], BF16, tag="hbT")
                for k in range(3):
                    pt = ptrp.tile([P, 128], BF16, tag="pt")
                    nc.tensor.transpose(
                        out=pt[:, :nb],
                        in_=h1_all[:nb, b * 384 + k * 128:b * 384 + (k + 1) * 128],
                        identity=identb[:nb, :nb])
                    nc.scalar.activation(
                        out=hbT[:, k * 128:k * 128 + nb], in_=pt[:, :nb],
                        func=AF.Relu, scale=sclT[:, k:k + 1], bias=shfT[:, k:k + 1])
                pz = pm.tile([P, 256], F32, tag="pmm")
                for k in range(3):
                    nc.tensor.matmul(out=pz[:nb, :],
                                     lhsT=hbT[:, k * 128:k * 128 + nb],
                                     rhs=w2h_sb[k][:], start=(k == 0), stop=(k == 2))
                zt = wk.tile([P, 256], BF16, tag="yb")
                nc.vector.tensor_tensor(out=zt[:nb, :], in0=pz[:nb, :],
                                        in1=bcast["b2h"][:nb, :], op=AL.add)
                nc.sync.dma_start(out=z12_loc[b * BLK:b * BLK + nb, :], in_=zt[:nb, :])
                p0 = pm.tile([P, 256], F32, tag="pmm")
                for k in range(3):
                    nc.tensor.matmul(out=p0[:nb, :128],
                                     lhsT=hbT[:, k * 128:k * 128 + nb],
                                     rhs=w20_sb[k][:], start=(k == 0), stop=(k == 2))
                nc.vector.tensor_tensor(out=h2a(b, nb),
                                        in0=p0[:nb, :128], in1=bcast["b20"][:nb, :],
                                        op=AL.add)
            if _on('dense'):
                nc.gpsimd.collective_compute(
                    "AllGather", AL.bypass, replica_groups=groups,
                    ins=[z12_loc[:]], outs=[z12_full[:]])

            # ---------------- layer 2, first + second applications
            if _on('a3'):
                dL, dH = mk_first_app(h2b, None, tpa_loc)
                spmm_pass("a3", z12_full, aidx, m_a, sched_a, 256, dL, dH)
                nc.gpsimd.collective_compute(
                    "AllGather", AL.bypass, replica_groups=groups,
                    ins=[tpa_loc[:]], outs=[tpa_full[:]])
            if _on('n3'):
                dL, dH = mk_first_app(h2b, LAM, tpn_loc)
                spmm_pass("n3", z12_full, nidx, m_nd, sched_n, 256, dL, dH)
                nc.gpsimd.collective_compute(
                    "AllGather", AL.bypass, replica_groups=groups,
                    ins=[tpn_loc[:]], outs=[tpn_full[:]])

            if _on('a4'):
                dL, dH = mk_second_app(h2c, True)
                spmm_pass("a4", tpa_full, aidx, m_a, sched_a, 128, dL, dH)

            def final_extra(b):
                nb = nb_of(b)
                hbT = wk.tile([P, 384], BF16, tag="hbT")
                for k, hsrc in enumerate((h2a, h2b, h2c)):
                    pt = ptrp.tile([P, 128], BF16, tag="pt")
                    nc.tensor.transpose(
                        out=pt[:, :nb], in_=hsrc(b, nb),
                        identity=identb[:nb, :nb])
                    nc.scalar.activation(out=hbT[:, k * 128:k * 128 + nb],
                                         in_=pt[:, :nb], func=AF.Copy)
                po = pm.tile([P, 256], F32, tag="pmm")
                for k in range(3):
                    nc.tensor.matmul(out=po[:nb, :128],
                                     lhsT=hbT[:, k * 128:k * 128 + nb],
                                     rhs=fpw_sb[k][:], start=(k == 0),
                                     stop=(k == 2))
                osb = wk.tile([P, 128], F32, tag="osb")
                nc.vector.tensor_tensor(out=osb[:nb, :], in0=po[:nb, :128],
                                        in1=bcast["fpb"][:nb, :], op=AL.add)
                nc.sync.dma_start(out=out_ext[b * BLK:b * BLK + nb, :],
                                  in_=osb[:nb, :])

            if _on('n4'):
                dL, dH = mk_second_app(h2c, False, extra=final_extra)
                spmm_pass("n4", tpn_full, nidx, m_ndl, sched_n, 128, dL, dH)
            if STOP_AFTER is not None:
                dz = wk.tile([P, 128], F32, tag="osb")
                nc.vector.memset(dz[:], 0.0)
                for b in range(NB):
                    nb = nb_of(b)
                    nc.sync.dma_start(out=out_ext[b * BLK:b * BLK + nb, :],
                                      in_=dz[:nb, :])

    return nc


def _make(x, val, nd_val,
          l1_W0, l1_b0, l1_W1, l1_b1, l1_W2, l1_b2,
          l2_W0, l2_b0, l2_W1, l2_b1, l2_W2, l2_b2,
          bn_gamma, bn_beta, fp_W, fp_b,
          row, col, nd_row, nd_col):
    import ml_dtypes
    x = np.asarray(x, np.float32)
    row = np.asarray(row, np.int64); col = np.asarray(col, np.int64)
    val = np.asarray(val, np.float32)
    nd_row = np.asarray(nd_row, np.int64); nd_col = np.asarray(nd_col, np.int64)
    nd_val = np.asarray(nd_val, np.float32)

    a_i, a_lr, a_vf, sched_a, TA = _prep_adj(row, col, val)
    n_i, n_lr, n_vf, sched_n, TN = _prep_adj(nd_row, nd_col, nd_val)
    m_a = _mhost(a_lr, a_vf, TA)
    m_nd = _mhost(n_lr, n_vf, TN)
    m_ndl = _mhost(n_lr, n_vf, TN, scale=LAM)

    bf = ml_dtypes.bfloat16
    xt = np.ascontiguousarray(x.T).astype(bf)                # [128, N]
    ident = np.eye(P, dtype=np.float32).astype(bf)
    ones1 = np.ones((1, P), np.float32)
    onesb = np.ones((P, 1), bf)
    w1h = np.ascontiguousarray(
        np.concatenate([l1_W1, l1_W2], 0).T).astype(bf)      # [128, 256]
    w10 = np.ascontiguousarray(
        (1.0 + LAM) * np.asarray(l1_W0, np.float32).T).astype(bf)
    w2h = np.ascontiguousarray(
        np.concatenate([l2_W1, l2_W2], 0).T).astype(bf)      # [384, 256]
    w20 = np.ascontiguousarray(
        (1.0 + LAM) * np.asarray(l2_W0, np.float32).T).astype(bf)
    fpw = np.ascontiguousarray(np.asarray(fp_W, np.float32).T).astype(bf)
    b2h = np.concatenate([l2_b1, l2_b2])[None, :].astype(np.float32)
    b20 = ((1.0 + LAM) * np.asarray(l2_b0))[None, :].astype(np.float32)
    fpb = np.asarray(fp_b)[None, :].astype(np.float32)
    bng = np.asarray(bn_gamma)[None, :].astype(np.float32)
    bnb = np.asarray(bn_beta)[None, :].astype(np.float32)

    nc = _build(TA, TN, sched_a, sched_n)
    # raw Bass skips Bacc's extended-inst codegen pass; without it the NEFF
    # compiler sees empty .instr on ISA subclasses -> "ISA wrong length"
    mybir.codegen_inst_isa_subclasses(nc)
    if FIX_WAITS:
        fix_waits(nc)

    in_maps = []
    for c in range(NCORES):
        in_maps.append({
            "xl": np.ascontiguousarray(xt[:, c * R:(c + 1) * R]),
            "a_idx": a_i[c], "n_idx": n_i[c],
            "m_a": m_a[c], "m_nd": m_nd[c], "m_ndl": m_ndl[c],
            "ident": ident, "ones1": ones1, "onesb": onesb,
            "w1h": w1h, "w10": w10, "w2h": w2h, "w20": w20, "fpw": fpw,
            "b2h": b2h, "b20": b20, "fpb": fpb, "bng": bng, "bnb": bnb,
        })
    return nc, in_maps


def kernel(**inputs):
    nc, in_maps = _make(**inputs)
    res = run_bass_kernel_spmd(nc, in_maps, list(range(NCORES)), trace=TRACE)
    LAST_RESULT["res"] = res
    out = np.concatenate([res.results[c]["out"] for c in range(NCORES)], axis=0)
    return out


# revision 6
# speedup vs baseline: 1.8476x; 1.0137x over previous
"""MixHop GNN (2 layers, 2 adjacencies, hops 0..2) on 8 trn2 NeuronCores.

Sharding: nodes row-partitioned across 8 cores (6250 rows each). Each SpMM
is computed for the core's destination rows only, gathering source rows
from a replicated DRAM table (AllGather between phases). The SpMM maps to
TensorE as a segment matmul: gather 128-edge tiles of source rows and
accumulate M^T @ G into a PSUM block of 128 destination rows, where
M[e, d] = val[e] * (lrow[e] == d).

v3 changes vs v2:
  * dma_gather descriptor prep (the v2 bottleneck: ~7.4ns/descriptor on Q7
    cores 0-1) is spread across all four SWDGE queues -- queue q's prep runs
    on Q7 cores 2q/2q+1, so round-robin queue assignment runs 4 preps
    concurrently (HW-measured 3.35x).
  * The val-scaled one-hot M matrices are precomputed on the host and
    streamed from DRAM (HWDGE, sequential 2KB/partition windows) instead of
    being built per-tile on VectorE (measured ~820ns/tile fixed-overhead
    floor -> 5.6ms total). A-passes share one M image; ND first/second
    applications use unscaled/lambda-scaled images so second-hop drains are
    plain adds.
  * BN-stats squaring moved to ScalarE (Square activation).

Self-contained: only numpy + ml_dtypes + concourse (environment packages).
"""
import numpy as np

import concourse.bass as bass
from concourse import mybir
from concourse.bass_utils import run_bass_kernel_spmd
from concourse.library_config import mlp
from concourse.tile import TileContext

F32 = mybir.dt.float32
BF16 = mybir.dt.bfloat16
I16 = mybir.dt.int16
AL = mybir.AluOpType
AF = mybir.ActivationFunctionType

N = 50000
NCORES = 8
R = N // NCORES          # 6250 rows per core
BLK = 128
NB = (R + BLK - 1) // BLK  # 49 blocks (48 full + 106)
LAM = 0.5
EPS = 1e-5
P = 128
HALF = 32768             # int16 gather-index limit
WIN = 8                  # 128-edge tiles per dma_gather call (1024 idxs =
                         # one ring's worth of descriptors; WIN=16 overflows
                         # the per-queue SWDGE ring and hangs)
NQ = 4                   # SWDGE queues; queue q's descriptor prep runs on Q7
                         # cores 2q/2q+1, so round-robin across 4 queues runs
                         # four preps concurrently instead of serializing on
                         # cores 0-1

TRACE = False            # set by test harness for profiling runs
FIX_WAITS = True         # disable for CoreSim validation (sim rejects
                         # post-hoc sync_info edits)
STOP_AFTER = None        # debug: truncate pipeline after a named stage
_STAGES = ['y12', 'hop0', 'a1', 'n1', 'a2', 'n2', 'bn', 'dense',
           'a3', 'n3', 'a4', 'n4']


def _on(stage):
    if STOP_AFTER is None:
        return True
    return _STAGES.index(stage) <= _STAGES.index(STOP_AFTER)
LAST_RESULT = {}


# ---------------------------------------------------------------- BIR post-pass
ASYNC_OPCODES = {"DMACopy", "CollectiveCompute", "DMAGatherAnt",
                 "DMAScatterAddAnt", "DMATransposeAnt"}


def _cap(inst) -> int:
    if inst.opcode in ("EventSemaphore", "NoOp"):
        return 999
    return 1


def fix_waits(nc, verbose=False):
    # --- collect streams (blocks concatenated in listed order; Tile output
    # is straight-line per engine)
    all_bbs = [bb for fn in nc.m.functions for bb in fn.blocks]
    streams = {}
    for bb in all_bbs:
        for inst in bb.instructions:
            streams.setdefault(inst.engine, []).append(inst)

    unsafe = set()
    wait_list = {}
    upd_list = {}
    for eng, insts in streams.items():
        for inst in insts:
            si = inst.sync_info
            ws, us = [], []
            if si:
                for w in (si.on_wait or []):
                    if getattr(w, "wait_mode", None) == "sem-ge-imm" and isinstance(
                            getattr(w, "wait_value", None), int):
                        ws.append((w.id, w.wait_value, w))
                    else:
                        ws.append((w.id, None, w))
                        unsafe.add(w.id)
                for u in (si.on_update or []):
                    um = getattr(u, "update_mode", None)
                    uv = getattr(u, "update_value", None)
                    if um == "sem-add-imm" and isinstance(uv, int):
                        us.append((u.id, uv))
                    elif um == "sem-inc":
                        us.append((u.id, 1))
                    else:
                        us.append((u.id, 0))
                        unsafe.add(u.id)
            wait_list[id(inst)] = ws
            upd_list[id(inst)] = us

    engines = list(streams.keys())
    ptr = {e: 0 for e in engines}
    vc = {e: {} for e in engines}
    sem_level = {}
    sem_cums = {}
    sem_snaps = {}

    def knowledge(s, v):
        cums = sem_cums.get(s)
        if not cums:
            return None
        import bisect
        i = bisect.bisect_left(cums, v)
        if i >= len(cums):
            i = len(cums) - 1
        return sem_snaps[s][i]

    n_dropped = 0
    progressed = True
    while progressed:
        progressed = False
        for eng in engines:
            insts = streams[eng]
            while ptr[eng] < len(insts):
                inst = insts[ptr[eng]]
                ws = wait_list[id(inst)]
                ok = True
                for (s, v, w) in ws:
                    if s in unsafe or v is None:
                        continue
                    if sem_level.get(s, 0) < v:
                        ok = False
                        break
                if not ok:
                    break
                myvc = vc[eng]
                kept = []
                # engine sems first: their knowledge usually implies the
                # DMA-lane waits, letting us drop the latter
                ws = sorted(ws, key=lambda t: str(
                    getattr(t[2], "ant_name", "")).startswith("DMA"))
                for (s, v, w) in ws:
                    if s not in unsafe and v is not None and myvc.get(s, 0) >= v:
                        n_dropped += 1
                        continue
                    kept.append(w)
                    if s in unsafe or v is None:
                        continue
                    k = knowledge(s, v)
                    if k:
                        for ks, kv in k.items():
                            if myvc.get(ks, 0) < kv:
                                myvc[ks] = kv
                    if myvc.get(s, 0) < v:
                        myvc[s] = v
                si = inst.sync_info
                if si and len(kept) != len(si.on_wait or []):
                    inst.sync_info = mybir.SyncInfo(
                        on_wait=kept, on_update=list(si.on_update or []))
                us = upd_list[id(inst)]
                if us:
                    is_async = inst.opcode in ASYNC_OPCODES
                    for (s, u) in us:
                        lvl = sem_level.get(s, 0) + u
                        sem_level[s] = lvl
                        if s not in unsafe:
                            snap = dict(myvc)
                            snap[s] = lvl
                            cums = sem_cums.setdefault(s, [])
                            snaps = sem_snaps.setdefault(s, [])
                            if snaps:
                                prev = snaps[-1]
                                for ks, kv in prev.items():
                                    if snap.get(ks, 0) < kv:
                                        snap[ks] = kv
                            cums.append(lvl)
                            snaps.append(snap)
                            if not is_async:
                                myvc[s] = lvl
                ptr[eng] += 1
                progressed = True

    stuck = sum(len(streams[e]) - ptr[e] for e in engines)
    # --- cap remaining waits with carriers
    uid = 0
    n_carriers = 0
    for bb in all_bbs:
        new_insts = []
        for inst in bb.instructions:
            si = inst.sync_info
            waits = list(si.on_wait) if (si and si.on_wait) else []
            cap = _cap(inst)
            if len(waits) > cap:
                keep = waits[len(waits) - cap:]
                for w in waits[: len(waits) - cap]:
                    uid += 1
                    new_insts.append(mybir.InstEventSemaphore(
                        name=f"waitfix-{uid}",
                        engine=inst.engine, ins=[], outs=[],
                        sync_info=mybir.SyncInfo(on_wait=[w], on_update=[]),
                    ))
                    n_carriers += 1
                inst.sync_info = mybir.SyncInfo(
                    on_wait=keep, on_update=list(si.on_update or []))
            new_insts.append(inst)
        bb.instructions = new_insts
    if verbose:
        print(f"fix_waits: dropped {n_dropped} redundant waits, "
              f"{n_carriers} carriers, {stuck} unprocessed")
    return nc


# ---------------------------------------------------------------- preprocessing
def _prep_adj(row, col, val):
    """Partition edges by destination core / 128-row block, then split each
    block's edges by source half (col < HALF vs >=) for int16 gather
    indices. Tile layout per pass: [all blocks' L tiles][all blocks' H
    tiles]; gather calls cover WIN-tile windows of each region.

    Returns:
      idx16 [NCORES, 128, T*8] int16  gather indices (wrapped 16-partition
                                      layout, replicated 8x)
      lrow_flat [NCORES, T*128]       local dest row per edge slot
      val_flat  [NCORES, T*128] f32   edge weight per slot (0 = pad)
      sched (TL, TH, [(b, lt0, kL, ht0, kH), ...])
      T = TL + TH total 128-edge tiles
    """
    row = np.asarray(row); col = np.asarray(col); val = np.asarray(val)
    core = row // R
    rloc = row - core * R
    blk = rloc // BLK
    lrow = rloc - blk * BLK
    half = (col >= HALF).astype(np.int64)
    idxval = np.where(half == 1, col - HALF, col).astype(np.int64)

    cnt = np.zeros((NCORES, NB, 2), np.int64)
    np.add.at(cnt, (core, blk, half), 1)
    kL = np.maximum(1, -(-cnt[:, :, 0].max(axis=0) // BLK))  # [NB]
    kH = np.maximum(1, -(-cnt[:, :, 1].max(axis=0) // BLK))  # [NB]

    lt0 = np.concatenate([[0], np.cumsum(kL)])
    TL = int(lt0[-1])
    ht0 = TL + np.concatenate([[0], np.cumsum(kH)])
    T = int(ht0[-1])
    sched = (TL, T - TL,
             [(b, int(lt0[b]), int(kL[b]), int(ht0[b]), int(kH[b]))
              for b in range(NB)])

    idx_flat = np.zeros((NCORES, T * BLK), np.int64)
    lrow_flat = np.zeros((NCORES, T * BLK), np.int64)
    val_flat = np.zeros((NCORES, T * BLK), np.float32)

    order = np.lexsort((col, half, blk, core))
    core_s, blk_s, half_s = core[order], blk[order], half[order]
    idx_s, lrow_s, val_s = idxval[order], lrow[order], val[order]
    key = (core_s * NB + blk_s) * 2 + half_s
    grid = np.arange(NCORES * NB * 2)
    starts = np.searchsorted(key, grid)
    ends = np.searchsorted(key, grid + 1)
    for c in range(NCORES):
        for b in range(NB):
            for h, base in ((0, lt0[b]), (1, ht0[b])):
                g = (c * NB + b) * 2 + h
                s, e = starts[g], ends[g]
                n = e - s
                if n == 0:
                    continue
                off = int(base) * BLK
                idx_flat[c, off:off + n] = idx_s[s:e]
                lrow_flat[c, off:off + n] = lrow_s[s:e]
                val_flat[c, off:off + n] = val_s[s:e]

    # idx wrap: call-local position i = t*128 + p lives at partition i%16,
    # column i//16 = t*8 + p//16 (call starts are tile-aligned, 128%16==0).
    arr = idx_flat.reshape(NCORES, T, 8, 16)           # [c, t, p//16, p%16]
    idx16 = arr.transpose(0, 3, 1, 2).reshape(NCORES, 16, T * 8)
    idx16 = np.ascontiguousarray(
        np.tile(idx16, (1, 8, 1)).astype(np.int16))    # [c, 128, T*8]
    return idx16, lrow_flat, val_flat, sched, T


def _mhost(lrow_flat, val_flat, T, scale=1.0):
    """Host-built M images: M[c][p, t*128 + j] = scale*val of the edge at
    (tile t, slot p) if lrow == j else 0.  [NCORES, 128, T*128] bf16."""
    import ml_dtypes
    M = np.zeros((NCORES, P, T * P), ml_dtypes.bfloat16)
    pos = np.arange(T * BLK)
    t = pos // BLK
    p = pos % BLK
    col = t * P + lrow_flat                            # [NCORES, T*128]
    v = (scale * val_flat).astype(ml_dtypes.bfloat16)
    for c in range(NCORES):
        M[c, p, col[c]] = v[c]
    return M


# ---------------------------------------------------------------- device program
def _build(TA, TN, sched_a, sched_n):
    nc = bass.Bass(num_devices=NCORES, num_swdge_queues=NQ,
                   dynamic_dma_scratch_size=32768)
    groups = [list(range(NCORES))]

    # ---- external I/O
    xl_in = nc.declare_dram_parameter("xl", [P, R], BF16, isOutput=False)
    a_idx = nc.declare_dram_parameter("a_idx", [P, TA * 8], I16, isOutput=False)
    n_idx = nc.declare_dram_parameter("n_idx", [P, TN * 8], I16, isOutput=False)
    m_a = nc.declare_dram_parameter("m_a", [P, TA * P], BF16, isOutput=False)
    m_nd = nc.declare_dram_parameter("m_nd", [P, TN * P], BF16, isOutput=False)
    m_ndl = nc.declare_dram_parameter("m_ndl", [P, TN * P], BF16, isOutput=False)
    ident_in = nc.declare_dram_parameter("ident", [P, P], BF16, isOutput=False)
    ones1_in = nc.declare_dram_parameter("ones1", [1, P], F32, isOutput=False)
    onesb_in = nc.declare_dram_parameter("onesb", [P, 1], BF16, isOutput=False)
    w1h_in = nc.declare_dram_parameter("w1h", [128, 256], BF16, isOutput=False)
    w10_in = nc.declare_dram_parameter("w10", [128, 128], BF16, isOutput=False)
    w2h_in = nc.declare_dram_parameter("w2h", [384, 256], BF16, isOutput=False)
    w20_in = nc.declare_dram_parameter("w20", [384, 128], BF16, isOutput=False)
    fpw_in = nc.declare_dram_parameter("fpw", [384, 128], BF16, isOutput=False)
    b2h_in = nc.declare_dram_parameter("b2h", [1, 256], F32, isOutput=False)
    b20_in = nc.declare_dram_parameter("b20", [1, 128], F32, isOutput=False)
    fpb_in = nc.declare_dram_parameter("fpb", [1, 128], F32, isOutput=False)
    bng_in = nc.declare_dram_parameter("bng", [1, 384], F32, isOutput=False)
    bnb_in = nc.declare_dram_parameter("bnb", [1, 384], F32, isOutput=False)
    out_ext = nc.declare_dram_parameter("out", [R, 128], F32, isOutput=True)

    with TileContext(nc) as tc:
        with (
            tc.tile_pool(name="consts", bufs=1) as consts,
            tc.tile_pool(name="glp", bufs=8) as glp,
            tc.tile_pool(name="mwp", bufs=8) as mwp,
            tc.tile_pool(name="wk", bufs=4) as wk,
            tc.tile_pool(name="ps", bufs=3, space="PSUM") as ps,
            tc.tile_pool(name="pm", bufs=1, space="PSUM") as pm,
            tc.tile_pool(name="ptr", bufs=2, space="PSUM") as ptrp,
            tc.tile_pool(name="pst", bufs=1, space="PSUM") as pstp,
            tc.tile_pool(name="dram", bufs=1, space="DRAM") as dram,
        ):
            nc.gpsimd.load_library(mlp)

            # num_idxs register cache: Pool has ~46 free registers and
            # to_reg() does not dedupe constants
            _regs = {}

            def nreg(v):
                if v not in _regs:
                    _regs[v] = nc.gpsimd.to_reg(v)
                return _regs[v]

            _qc = [0]

            def next_q():
                q = _qc[0]
                _qc[0] = (q + 1) % NQ
                return q

            # ---------------- DRAM scratch
            y12_loc = dram.tile([R, 256], BF16)
            ta_loc = dram.tile([R, 128], BF16)
            tn_loc = dram.tile([R, 128], BF16)
            z12_loc = dram.tile([R, 256], BF16)
            tpa_loc = dram.tile([R, 128], BF16)
            tpn_loc = dram.tile([R, 128], BF16)
            bn_loc = dram.tile([1, 768], F32)
            ss_dram = dram.tile([6, 128], F32)
            y12_full = dram.tile([N, 256], BF16, addr_space="Shared")
            ta_full = dram.tile([N, 128], BF16, addr_space="Shared")
            tn_full = dram.tile([N, 128], BF16, addr_space="Shared")
            z12_full = dram.tile([N, 256], BF16, addr_space="Shared")
            tpa_full = dram.tile([N, 128], BF16, addr_space="Shared")
            tpn_full = dram.tile([N, 128], BF16, addr_space="Shared")
            bn_full = dram.tile([1, 768], F32, addr_space="Shared")

            # ---------------- constants to SBUF
            def cload(src, shape, dtype):
                t = consts.tile(shape, dtype, name=f"c_{src.name}")
                nc.sync.dma_start(out=t[:], in_=src[:])
                return t

            xl = cload(xl_in, [P, R], BF16)
            identb = cload(ident_in, [P, P], BF16)
            ones1 = cload(ones1_in, [1, P], F32)
            onesb = cload(onesb_in, [P, 1], BF16)
            w1h = cload(w1h_in, [128, 256], BF16)
            w10 = cload(w10_in, [128, 128], BF16)
            b2h = cload(b2h_in, [1, 256], F32)
            b20 = cload(b20_in, [1, 128], F32)
            fpb = cload(fpb_in, [1, 128], F32)
            bng = cload(bng_in, [1, 384], F32)
            bnb = cload(bnb_in, [1, 384], F32)
            w2h_sb, w20_sb, fpw_sb = [], [], []
            for k in range(3):
                t = consts.tile([128, 256], BF16, name=f"w2h{k}")
                nc.sync.dma_start(out=t[:], in_=w2h_in[k * 128:(k + 1) * 128, :])
                w2h_sb.append(t)
                t = consts.tile([128, 128], BF16, name=f"w20{k}")
                nc.sync.dma_start(out=t[:], in_=w20_in[k * 128:(k + 1) * 128, :])
                w20_sb.append(t)
                t = consts.tile([128, 128], BF16, name=f"fpw{k}")
                nc.sync.dma_start(out=t[:], in_=fpw_in[k * 128:(k + 1) * 128, :])
                fpw_sb.append(t)

            aidx = cload(a_idx, [P, TA * 8], I16)
            nidx = cload(n_idx, [P, TN * 8], I16)

            # persistent per-node-block SBUF arrays. Layer-2's three hop
            # slices reuse h1_all's storage: block b's layer-1 features die
            # exactly when its layer-2 values are produced (hop0' is written
            # after the stage-7 transposes read the block; hop1'/hop2' are
            # written in later passes).
            h1_all = consts.tile([P, NB * 384], BF16, name="h1_all")
            sclT = consts.tile([P, 3], F32, name="sclT")
            shfT = consts.tile([P, 3], F32, name="shfT")

            def h2a(b, nb=P):
                return h1_all[:nb, b * 384:b * 384 + 128]

            def h2b(b, nb=P):
                return h1_all[:nb, b * 384 + 128:b * 384 + 256]

            def h2c(b, nb=P):
                return h1_all[:nb, b * 384 + 256:b * 384 + 384]

            def nb_of(b):
                return BLK if b < NB - 1 else R - BLK * (NB - 1)

            # broadcast bias rows to all partitions (b2h, b20*1.5, fpb)
            bcast = {}
            for nm, bsrc, wdt in (("b2h", b2h, 256), ("b20", b20, 128),
                                  ("fpb", fpb, 128)):
                pbx = pm.tile([P, 256], F32, tag="pmm", name=f"pb_{nm}")
                nc.tensor.matmul(out=pbx[:, :wdt], lhsT=ones1[:], rhs=bsrc[:],
                                 start=True, stop=True)
                bt = consts.tile([P, wdt], F32, name=f"bb_{nm}")
                nc.vector.tensor_copy(out=bt[:], in_=pbx[:, :wdt])
                bcast[nm] = bt

            # ---------------- stage 1: local Y12 = X @ [W1;W2]^T (no bias:
            # layer-1 biases cancel in BatchNorm), then AllGather.
            for b in range(NB) if _on('y12') else []:
                nb = nb_of(b)
                py = pm.tile([P, 256], F32, tag="pmm")
                nc.tensor.matmul(out=py[:nb, :], lhsT=xl[:, b * BLK:b * BLK + nb],
                                 rhs=w1h[:], start=True, stop=True)
                yb = wk.tile([P, 256], BF16, tag="yb")
                nc.scalar.activation(out=yb[:nb, :], in_=py[:nb, :], func=AF.Copy)
                nc.sync.dma_start(out=y12_loc[b * BLK:b * BLK + nb, :],
                                  in_=yb[:nb, :])
            if _on('y12'):
                nc.gpsimd.collective_compute(
                    "AllGather", AL.bypass, replica_groups=groups,
                    ins=[y12_loc[:]], outs=[y12_full[:]])

            # hop0 while the AllGather runs: h1 hop0 = 1.5 * x @ W0^T
            # (w10 pre-scaled by 1.5 on host; bias cancels in BN)
            for b in range(NB) if _on('hop0') else []:
                nb = nb_of(b)
                p0 = pm.tile([P, 256], F32, tag="pmm")
                nc.tensor.matmul(out=p0[:nb, :128], lhsT=xl[:, b * BLK:b * BLK + nb],
                                 rhs=w10[:], start=True, stop=True)
                nc.scalar.activation(out=h1_all[:nb, b * 384:b * 384 + 128],
                                     in_=p0[:nb, :128], func=AF.Copy)

            # ---------------- batched-gather SpMM pass: L phase over the
            # low table half, then H phase over the high half (int16 gather
            # indices). Gather calls cover WIN-tile windows, round-robin
            # across the 4 SWDGE queues; the matching M window streams in
            # via HWDGE. Each block accumulates one PSUM per phase; the H
            # drain combines with the L result.
            tstage = consts.tile([P, NB * 128], BF16, name="tstage")

            def spmm_phase(pfx, table_half, idxT, mdram, t0, ntiles,
                           binfo, elem, drain):
                wins = []
                mwins = []
                for wi, w0 in enumerate(range(0, ntiles, WIN)):
                    n = min(WIN, ntiles - w0)
                    g = glp.tile([P, WIN * elem], BF16, tag="gw",
                                 name=f"{pfx}g{w0}")
                    nc.gpsimd.dma_gather(
                        out_ap=g[:, :n * elem].rearrange(
                            "p (t e) -> p t e", e=elem),
                        in_ap=table_half,
                        idxs_ap=idxT[:, (t0 + w0) * 8:(t0 + w0 + n) * 8],
                        num_idxs=n * BLK, num_idxs_reg=nreg(n * BLK),
                        elem_size=elem, queue_num=next_q())
                    wins.append(g)
                    mw = mwp.tile([P, WIN * P], BF16, tag="mw",
                                  name=f"{pfx}m{w0}")
                    eng = nc.sync if wi % 2 == 0 else nc.scalar
                    eng.dma_start(out=mw[:, :n * P],
                                  in_=mdram[:, (t0 + w0) * P:(t0 + w0 + n) * P])
                    mwins.append(mw)
                for (b, bt0, k) in binfo:
                    psum = ps.tile([P, elem], F32, tag="sp", name=f"{pfx}ps{b}")
                    for i in range(k):
                        t = bt0 + i
                        lt = t - t0
                        g = wins[lt // WIN]
                        mw = mwins[lt // WIN]
                        s = lt % WIN
                        nc.tensor.matmul(
                            out=psum[:], lhsT=mw[:, s * P:(s + 1) * P],
                            rhs=g[:, s * elem:(s + 1) * elem],
                            start=(i == 0), stop=(i == k - 1))
                    drain(b, psum)

            def spmm_pass(pfx, table, idxT, mdram, sched, elem,
                          drainL, drainH):
                TL, TH, blocks = sched
                spmm_phase(pfx + "L", table[0:HALF, :], idxT, mdram,
                           0, TL,
                           [(b, lt0, kL) for (b, lt0, kL, _, _) in blocks],
                           elem, drainL)
                spmm_phase(pfx + "H", table[HALF:N, :], idxT, mdram,
                           TL, TH,
                           [(b, ht0, kH) for (b, _, _, ht0, kH) in blocks],
                           elem, drainH)

            # ---------------- layer 1, first application (hop1 + T tables)
            def mk_first_app(hslice, lam_first, t_loc):
                # hslice(b) -> target AP for the hop-1 slice; lam_first:
                # None for the A pass (copy), LAM for the ND pass (l-add)
                def dL(b, p):
                    sl = hslice(b)
                    if lam_first is None:
                        nc.vector.tensor_copy(out=sl, in_=p[:, 0:128])
                    else:
                        nc.vector.scalar_tensor_tensor(
                            out=sl, in0=p[:, 0:128], scalar=lam_first, in1=sl,
                            op0=AL.mult, op1=AL.add)
                    nc.scalar.activation(out=tstage[:, b * 128:(b + 1) * 128],
                                         in_=p[:, 128:256], func=AF.Copy)

                def dH(b, p):
                    nb = nb_of(b)
                    sl = hslice(b)
                    if lam_first is None:
                        nc.vector.tensor_tensor(out=sl, in0=p[:, 0:128],
                                                in1=sl, op=AL.add)
                    else:
                        nc.vector.scalar_tensor_tensor(
                            out=sl, in0=p[:, 0:128], scalar=lam_first, in1=sl,
                            op0=AL.mult, op1=AL.add)
                    tsb = wk.tile([P, 128], BF16, tag="tsb")
                    nc.vector.tensor_tensor(
                        out=tsb[:], in0=p[:, 128:256],
                        in1=tstage[:, b * 128:(b + 1) * 128], op=AL.add)
                    nc.sync.dma_start(out=t_loc[b * BLK:b * BLK + nb, :],
                                      in_=tsb[:nb, :])
                return dL, dH

            def h1_hop1(b, nb=P):
                return h1_all[:nb, b * 384 + 128:b * 384 + 256]

            def h1_hop2(b, nb=P):
                return h1_all[:nb, b * 384 + 256:b * 384 + 384]

            if _on('a1'):
                dL, dH = mk_first_app(h1_hop1, None, ta_loc)
                spmm_pass("a1", y12_full, aidx, m_a, sched_a, 256, dL, dH)
                nc.gpsimd.collective_compute(
                    "AllGather", AL.bypass, replica_groups=groups,
                    ins=[ta_loc[:]], outs=[ta_full[:]])
            if _on('n1'):
                dL, dH = mk_first_app(h1_hop1, LAM, tn_loc)
                spmm_pass("n1", y12_full, nidx, m_nd, sched_n, 256, dL, dH)
                nc.gpsimd.collective_compute(
                    "AllGather", AL.bypass, replica_groups=groups,
                    ins=[tn_loc[:]], outs=[tn_full[:]])

            # ---------------- layer 1, second application (hop2) + BN stats
            # ND second applications use the lambda-prescaled M image, so
            # every drain is a plain copy/add.
            def mk_second_app(hslice, init, extra=None):
                def dL(b, p):
                    sl = hslice(b)
                    if init:
                        nc.vector.tensor_copy(out=sl, in_=p[:, 0:128])
                    else:
                        nc.vector.tensor_tensor(out=sl, in0=p[:, 0:128],
                                                in1=sl, op=AL.add)

                def dH(b, p):
                    sl = hslice(b)
                    nc.vector.tensor_tensor(out=sl, in0=p[:, 0:128],
                                            in1=sl, op=AL.add)
                    if extra is not None:
                        extra(b)
                return dL, dH

            if _on('a2'):
                dL, dH = mk_second_app(h1_hop2, True)
                spmm_pass("a2", ta_full, aidx, m_a, sched_a, 128, dL, dH)

            pst = pstp.tile([1, 384], F32, tag="pst", name="pst")
            psq = pstp.tile([1, 384], F32, tag="psq", name="psq")

            def stats_extra(b):
                nb = nb_of(b)
                hsl = h1_all[:, b * 384:b * 384 + 384]
                sq = wk.tile([P, 384], BF16, tag="sq")
                nc.scalar.activation(out=sq[:nb, :], in_=hsl[:nb],
                                     func=AF.Square)
                nc.tensor.matmul(out=pst[:], lhsT=onesb[:nb, :], rhs=hsl[:nb],
                                 start=(b == 0), stop=(b == NB - 1))
                nc.tensor.matmul(out=psq[:], lhsT=onesb[:nb, :], rhs=sq[:nb, :],
                                 start=(b == 0), stop=(b == NB - 1))

            if _on('n2'):
                dL, dH = mk_second_app(h1_hop2, False, extra=stats_extra)
                spmm_pass("n2", tn_full, nidx, m_ndl, sched_n, 128, dL, dH)

            # ---------------- BN finalize (allreduce + feature-major params)
            if _on('bn'):
                stats = wk.tile([1, 768], F32, tag="bnst", bufs=1)
                nc.vector.tensor_copy(out=stats[:, 0:384], in_=pst[:])
                nc.vector.tensor_copy(out=stats[:, 384:768], in_=psq[:])
                nc.sync.dma_start(out=bn_loc[:], in_=stats[:])
                nc.gpsimd.collective_compute(
                    "AllReduce", AL.add, replica_groups=groups,
                    ins=[bn_loc[:]], outs=[bn_full[:]])
                bnr = wk.tile([1, 768], F32, tag="bnr", bufs=1)
                nc.sync.dma_start(out=bnr[:], in_=bn_full[:])
                mean = wk.tile([1, 384], F32, tag="bn1", bufs=1)
                var = wk.tile([1, 384], F32, tag="bn2", bufs=1)
                tmp = wk.tile([1, 384], F32, tag="bn3", bufs=1)
                sshf = wk.tile([1, 768], F32, tag="bn4", bufs=1)
                nc.vector.tensor_scalar(out=mean[:], in0=bnr[:, 0:384],
                                        scalar1=1.0 / N, scalar2=None, op0=AL.mult)
                nc.vector.tensor_scalar(out=var[:], in0=bnr[:, 384:768],
                                        scalar1=1.0 / N, scalar2=None, op0=AL.mult)
                nc.vector.tensor_tensor(out=tmp[:], in0=mean[:], in1=mean[:], op=AL.mult)
                nc.vector.tensor_tensor(out=var[:], in0=var[:], in1=tmp[:], op=AL.subtract)
                nc.vector.tensor_scalar(out=var[:], in0=var[:], scalar1=EPS,
                                        scalar2=None, op0=AL.add)
                nc.scalar.sqrt(out=var[:], in_=var[:])
                nc.vector.reciprocal(out=var[:], in_=var[:])
                nc.vector.tensor_tensor(out=sshf[:, 0:384], in0=bng[:], in1=var[:],
                                        op=AL.mult)
                nc.vector.tensor_tensor(out=tmp[:], in0=mean[:], in1=sshf[:, 0:384],
                                        op=AL.mult)
                nc.vector.tensor_tensor(out=sshf[:, 384:768], in0=bnb[:], in1=tmp[:],
                                        op=AL.subtract)
                nc.sync.dma_start(out=ss_dram[:], in_=sshf[:])
                for k in range(3):
                    nc.sync.dma_start(out=sclT[:, k:k + 1], in_=ss_dram[k:k + 1, :])
                    nc.sync.dma_start(out=shfT[:, k:k + 1], in_=ss_dram[k + 3:k + 4, :])

            # ---------------- layer-2 dense: transpose, fused BN+relu (ACT),
            # linears; z12 local then AllGather
            for b in range(NB) if _on('dense') else []:
                nb = nb_of(b)
                hbT = wk.tile([P, 384], BF16, tag="hbT")
                for k in range(3):
                    pt = ptrp.tile([P, 128], BF16, tag="pt")
                    nc.tensor.transpose(
                        out=pt[:, :nb],
                        in_=h1_all[:nb, b * 384 + k * 128:b * 384 + (k + 1) * 128],
                        identity=identb[:nb, :nb])
                    nc.scalar.activation(
                        out=hbT[:, k * 128:k * 128 + nb], in_=pt[:, :nb],
                        func=AF.Relu, scale=sclT[:, k:k + 1], bias=shfT[:, k:k + 1])
                pz = pm.tile([P, 256], F32, tag="pmm")
                for k in range(3):
                    nc.tensor.matmul(out=pz[:nb, :],
                                     lhsT=hbT[:, k * 128:k * 128 + nb],
                                     rhs=w2h_sb[k][:], start=(k == 0), stop=(k == 2))
                zt = wk.tile([P, 256], BF16, tag="yb")
                nc.vector.tensor_tensor(out=zt[:nb, :], in0=pz[:nb, :],
                                        in1=bcast["b2h"][:nb, :], op=AL.add)
                nc.sync.dma_start(out=z12_loc[b * BLK:b * BLK + nb, :], in_=zt[:nb, :])
                p0 = pm.tile([P, 256], F32, tag="pmm")
                for k in range(3):
                    nc.tensor.matmul(out=p0[:nb, :128],
                                     lhsT=hbT[:, k * 128:k * 128 + nb],
                                     rhs=w20_sb[k][:], start=(k == 0), stop=(k == 2))
                nc.vector.tensor_tensor(out=h2a(b, nb),
                                        in0=p0[:nb, :128], in1=bcast["b20"][:nb, :],
                                        op=AL.add)
            if _on('dense'):
                nc.gpsimd.collective_compute(
                    "AllGather", AL.bypass, replica_groups=groups,
                    ins=[z12_loc[:]], outs=[z12_full[:]])

            # ---------------- layer 2, first + second applications
            if _on('a3'):
                dL, dH = mk_first_app(h2b, None, tpa_loc)
                spmm_pass("a3", z12_full, aidx, m_a, sched_a, 256, dL, dH)
                nc.gpsimd.collective_compute(
                    "AllGather", AL.bypass, replica_groups=groups,
                    ins=[tpa_loc[:]], outs=[tpa_full[:]])
            if _on('n3'):
                dL, dH = mk_first_app(h2b, LAM, tpn_loc)
                spmm_pass("n3", z12_full, nidx, m_nd, sched_n, 256, dL, dH)
                nc.gpsimd.collective_compute(
                    "AllGather", AL.bypass, replica_groups=groups,
                    ins=[tpn_loc[:]], outs=[tpn_full[:]])

            if _on('a4'):
                dL, dH = mk_second_app(h2c, True)
                spmm_pass("a4", tpa_full, aidx, m_a, sched_a, 128, dL, dH)

            def final_extra(b):
                nb = nb_of(b)
                hbT = wk.tile([P, 384], BF16, tag="hbT")
                for k, hsrc in enumerate((h2a, h2b, h2c)):
                    pt = ptrp.tile([P, 128], BF16, tag="pt")
                    nc.tensor.transpose(
                        out=pt[:, :nb], in_=hsrc(b, nb),
                        identity=identb[:nb, :nb])
                    nc.scalar.activation(out=hbT[:, k * 128:k * 128 + nb],
                                         in_=pt[:, :nb], func=AF.Copy)
                po = pm.tile([P, 256], F32, tag="pmm")
                for k in range(3):
                    nc.tensor.matmul(out=po[:nb, :128],
                                     lhsT=hbT[:, k * 128:k * 128 + nb],
                                     rhs=fpw_sb[k][:], start=(k == 0),
                                     stop=(k == 2))
                osb = wk.tile([P, 128], F32, tag="osb")
                nc.vector.tensor_tensor(out=osb[:nb, :], in0=po[:nb, :128],
                                        in1=bcast["fpb"][:nb, :], op=AL.add)
                nc.sync.dma_start(out=out_ext[b * BLK:b * BLK + nb, :],
                                  in_=osb[:nb, :])

            if _on('n4'):
                dL, dH = mk_second_app(h2c, False, extra=final_extra)
                spmm_pass("n4", tpn_full, nidx, m_ndl, sched_n, 128, dL, dH)
            if STOP_AFTER is not None:
                dz = wk.tile([P, 128], F32, tag="osb")
                nc.vector.memset(dz[:], 0.0)
                for b in range(NB):
                    nb = nb_of(b)
                    nc.sync.dma_start(out=out_ext[b * BLK:b * BLK + nb, :],
                                      in_=dz[:nb, :])

    return nc


def _make(x, val, nd_val,
          l1_W0, l1_b0, l1_W1, l1_b1, l1_W2, l1_b2,
          l2_W0, l2_b0, l2_W1, l2_b1, l2_W2, l2_b2,
          bn_gamma, bn_beta, fp_W, fp_b,
          row, col, nd_row, nd_col):
    import ml_dtypes
    x = np.asarray(x, np.float32)
    row = np.asarray(row, np.int64); col = np.asarray(col, np.int64)
    val = np.asarray(val, np.float32)
    nd_row = np.asarray(nd_row, np.int64); nd_col = np.asarray(nd_col, np.int64)
    nd_val = np.asarray(nd_val, np.float32)

    a_i, a_lr, a_vf, sched_a, TA = _prep_adj(row, col, val)
    n_i, n_lr, n_vf, sched_n, TN = _prep_adj(nd_row, nd_col, nd_val)
    m_a = _mhost(a_lr, a_vf, TA)
    m_nd = _mhost(n_lr, n_vf, TN)
    m_ndl = _mhost(n_lr, n_vf, TN, scale=LAM)

    bf = ml_dtypes.bfloat16
    xt = np.ascontiguousarray(x.T).astype(bf)                # [128, N]
    ident = np.eye(P, dtype=np.float32).astype(bf)
    ones1 = np.ones((1, P), np.float32)
    onesb = np.ones((P, 1), bf)
    w1h = np.ascontiguousarray(
        np.concatenate([l1_W1, l1_W2], 0).T).astype(bf)      # [128, 256]
    w10 = np.ascontiguousarray(
        (1.0 + LAM) * np.asarray(l1_W0, np.float32).T).astype(bf)
    w2h = np.ascontiguousarray(
        np.concatenate([l2_W1, l2_W2], 0).T).astype(bf)      # [384, 256]
    w20 = np.ascontiguousarray(
        (1.0 + LAM) * np.asarray(l2_W0, np.float32).T).astype(bf)
    fpw = np.ascontiguousarray(np.asarray(fp_W, np.float32).T).astype(bf)
    b2h = np.concatenate([l2_b1, l2_b2])[None, :].astype(np.float32)
    b20 = ((1.0 + LAM) * np.asarray(l2_b0))[None, :].astype(np.float32)
    fpb = np.asarray(fp_b)[None, :].astype(np.float32)
    bng = np.asarray(bn_gamma)[None, :].astype(np.float32)
    bnb = np.asarray(bn_beta)[None, :].astype(np.float32)

    nc = _build(TA, TN, sched_a, sched_n)
    # raw Bass skips Bacc's extended-inst codegen pass; without it the NEFF
    # compiler sees empty .instr on ISA subclasses -> "ISA wrong length"
    mybir.codegen_inst_isa_subclasses(nc)
    if FIX_WAITS:
        fix_waits(nc)

    in_maps = []
    for c in range(NCORES):
        in_maps.append({
            "xl": np.ascontiguousarray(xt[:, c * R:(c + 1) * R]),
            "a_idx": a_i[c], "n_idx": n_i[c],
            "m_a": m_a[c], "m_nd": m_nd[c], "m_ndl": m_ndl[c],
            "ident": ident, "ones1": ones1, "onesb": onesb,
            "w1h": w1h, "w10": w10, "w2h": w2h, "w20": w20, "fpw": fpw,
            "b2h": b2h, "b20": b20, "fpb": fpb, "bng": bng, "bnb": bnb,
        })
    return nc, in_maps


def kernel(**inputs):
    nc, in_maps = _make(**inputs)
    res = run_bass_kernel_spmd(nc, in_maps, list(range(NCORES)), trace=TRACE)
    LAST_RESULT["res"] = res
    out = np.concatenate([res.results[c]["out"] for c in range(NCORES)], axis=0)
    return out


# revision 12
# speedup vs baseline: 1.9366x; 1.0481x over previous
"""MixHop GNN (2 layers, 2 adjacencies, hops 0..2) on 8 trn2 NeuronCores.

Sharding: nodes row-partitioned across 8 cores (6250 rows each). Each SpMM
is computed for the core's destination rows only, gathering source rows
from a replicated DRAM table (AllGather between phases). The SpMM maps to
TensorE as a segment matmul: gather 128-edge tiles of source rows and
accumulate M^T @ G into a PSUM block of 128 destination rows, where
M[e, d] = val[e] * (lrow[e] == d).

v3 changes vs v2:
  * dma_gather descriptor prep (the v2 bottleneck: ~7.4ns/descriptor on Q7
    cores 0-1) is spread across all four SWDGE queues -- queue q's prep runs
    on Q7 cores 2q/2q+1, so round-robin queue assignment runs 4 preps
    concurrently (HW-measured 3.35x).
  * The val-scaled one-hot M matrices are precomputed on the host and
    streamed from DRAM (HWDGE, sequential 2KB/partition windows) instead of
    being built per-tile on VectorE (measured ~820ns/tile fixed-overhead
    floor -> 5.6ms total). A-passes share one M image; ND first/second
    applications use unscaled/lambda-scaled images so second-hop drains are
    plain adds.
  * BN-stats squaring moved to ScalarE (Square activation).

Self-contained: only numpy + ml_dtypes + concourse (environment packages).
"""
import numpy as np

import concourse.bass as bass
from concourse import mybir
from concourse.bass_utils import run_bass_kernel_spmd
from concourse.library_config import mlp
from concourse.tile import TileContext

F32 = mybir.dt.float32
BF16 = mybir.dt.bfloat16
I16 = mybir.dt.int16
AL = mybir.AluOpType
AF = mybir.ActivationFunctionType

N = 50000
NCORES = 8
R = N // NCORES          # 6250 rows per core
BLK = 128
NB = (R + BLK - 1) // BLK  # 49 blocks (48 full + 106)
LAM = 0.5
EPS = 1e-5
P = 128
HALF = 32768             # int16 gather-index limit
WIN = 8                  # 128-edge tiles per dma_gather call (1024 idxs =
                         # one ring's worth of descriptors; WIN=16 overflows
                         # the per-queue SWDGE ring and hangs)
VDVE_OF = 2              # of every 5 windows, this many get their M tiles
                         # built on VectorE instead of streamed from DRAM
                         # (cuts M HBM traffic ~40%; DVE is otherwise idle)
NQ = 4                   # SWDGE queues; queue q's descriptor prep runs on Q7
                         # cores 2q/2q+1, so round-robin across 4 queues runs
                         # four preps concurrently instead of serializing on
                         # cores 0-1

TRACE = False            # set by test harness for profiling runs
FIX_WAITS = True         # disable for CoreSim validation (sim rejects
                         # post-hoc sync_info edits)
STOP_AFTER = None        # debug: truncate pipeline after a named stage
_STAGES = ['y12', 'hop0', 'a1', 'n1', 'a2', 'n2', 'bn', 'dense',
           'a3', 'n3', 'a4', 'n4']


def _on(stage):
    if STOP_AFTER is None:
        return True
    return _STAGES.index(stage) <= _STAGES.index(STOP_AFTER)
LAST_RESULT = {}


# ---------------------------------------------------------------- BIR post-pass
ASYNC_OPCODES = {"DMACopy", "CollectiveCompute", "DMAGatherAnt",
                 "DMAScatterAddAnt", "DMATransposeAnt"}


def _cap(inst) -> int:
    if inst.opcode in ("EventSemaphore", "NoOp"):
        return 999
    return 1


def fix_waits(nc, verbose=False):
    # --- collect streams (blocks concatenated in listed order; Tile output
    # is straight-line per engine)
    all_bbs = [bb for fn in nc.m.functions for bb in fn.blocks]
    streams = {}
    for bb in all_bbs:
        for inst in bb.instructions:
            streams.setdefault(inst.engine, []).append(inst)

    unsafe = set()
    wait_list = {}
    upd_list = {}
    for eng, insts in streams.items():
        for inst in insts:
            si = inst.sync_info
            ws, us = [], []
            if si:
                for w in (si.on_wait or []):
                    if getattr(w, "wait_mode", None) == "sem-ge-imm" and isinstance(
                            getattr(w, "wait_value", None), int):
                        ws.append((w.id, w.wait_value, w))
                    else:
                        ws.append((w.id, None, w))
                        unsafe.add(w.id)
                for u in (si.on_update or []):
                    um = getattr(u, "update_mode", None)
                    uv = getattr(u, "update_value", None)
                    if um == "sem-add-imm" and isinstance(uv, int):
                        us.append((u.id, uv))
                    elif um == "sem-inc":
                        us.append((u.id, 1))
                    else:
                        us.append((u.id, 0))
                        unsafe.add(u.id)
            wait_list[id(inst)] = ws
            upd_list[id(inst)] = us

    engines = list(streams.keys())
    ptr = {e: 0 for e in engines}
    vc = {e: {} for e in engines}
    sem_level = {}
    sem_cums = {}
    sem_snaps = {}

    def knowledge(s, v):
        cums = sem_cums.get(s)
        if not cums:
            return None
        import bisect
        i = bisect.bisect_left(cums, v)
        if i >= len(cums):
            i = len(cums) - 1
        return sem_snaps[s][i]

    n_dropped = 0
    progressed = True
    while progressed:
        progressed = False
        for eng in engines:
            insts = streams[eng]
            while ptr[eng] < len(insts):
                inst = insts[ptr[eng]]
                ws = wait_list[id(inst)]
                ok = True
                for (s, v, w) in ws:
                    if s in unsafe or v is None:
                        continue
                    if sem_level.get(s, 0) < v:
                        ok = False
                        break
                if not ok:
                    break
                myvc = vc[eng]
                kept = []
                # engine sems first: their knowledge usually implies the
                # DMA-lane waits, letting us drop the latter
                ws = sorted(ws, key=lambda t: str(
                    getattr(t[2], "ant_name", "")).startswith("DMA"))
                for (s, v, w) in ws:
                    if s not in unsafe and v is not None and myvc.get(s, 0) >= v:
                        n_dropped += 1
                        continue
                    kept.append(w)
                    if s in unsafe or v is None:
                        continue
                    k = knowledge(s, v)
                    if k:
                        for ks, kv in k.items():
                            if myvc.get(ks, 0) < kv:
                                myvc[ks] = kv
                    if myvc.get(s, 0) < v:
                        myvc[s] = v
                si = inst.sync_info
                if si and len(kept) != len(si.on_wait or []):
                    inst.sync_info = mybir.SyncInfo(
                        on_wait=kept, on_update=list(si.on_update or []))
                us = upd_list[id(inst)]
                if us:
                    is_async = inst.opcode in ASYNC_OPCODES
                    for (s, u) in us:
                        lvl = sem_level.get(s, 0) + u
                        sem_level[s] = lvl
                        if s not in unsafe:
                            snap = dict(myvc)
                            snap[s] = lvl
                            cums = sem_cums.setdefault(s, [])
                            snaps = sem_snaps.setdefault(s, [])
                            if snaps:
                                prev = snaps[-1]
                                for ks, kv in prev.items():
                                    if snap.get(ks, 0) < kv:
                                        snap[ks] = kv
                            cums.append(lvl)
                            snaps.append(snap)
                            if not is_async:
                                myvc[s] = lvl
                ptr[eng] += 1
                progressed = True

    stuck = sum(len(streams[e]) - ptr[e] for e in engines)
    # --- cap remaining waits with carriers
    uid = 0
    n_carriers = 0
    for bb in all_bbs:
        new_insts = []
        for inst in bb.instructions:
            si = inst.sync_info
            waits = list(si.on_wait) if (si and si.on_wait) else []
            cap = _cap(inst)
            if len(waits) > cap:
                keep = waits[len(waits) - cap:]
                for w in waits[: len(waits) - cap]:
                    uid += 1
                    new_insts.append(mybir.InstEventSemaphore(
                        name=f"waitfix-{uid}",
                        engine=inst.engine, ins=[], outs=[],
                        sync_info=mybir.SyncInfo(on_wait=[w], on_update=[]),
                    ))
                    n_carriers += 1
                inst.sync_info = mybir.SyncInfo(
                    on_wait=keep, on_update=list(si.on_update or []))
            new_insts.append(inst)
        bb.instructions = new_insts
    if verbose:
        print(f"fix_waits: dropped {n_dropped} redundant waits, "
              f"{n_carriers} carriers, {stuck} unprocessed")
    return nc


# ---------------------------------------------------------------- preprocessing
def _prep_adj(row, col, val):
    """Partition edges by destination core / 128-row block, then split each
    block's edges by source half (col < HALF vs >=) for int16 gather
    indices. Tile layout per pass: [all blocks' L tiles][all blocks' H
    tiles]; gather calls cover WIN-tile windows of each region.

    Returns:
      idx16 [NCORES, 128, T*8] int16  gather indices (wrapped 16-partition
                                      layout, replicated 8x)
      lrow_flat [NCORES, T*128]       local dest row per edge slot
      val_flat  [NCORES, T*128] f32   edge weight per slot (0 = pad)
      sched (TL, TH, [(b, lt0, kL, ht0, kH), ...])
      T = TL + TH total 128-edge tiles
    """
    row = np.asarray(row); col = np.asarray(col); val = np.asarray(val)
    core = row // R
    rloc = row - core * R
    blk = rloc // BLK
    lrow = rloc - blk * BLK
    half = (col >= HALF).astype(np.int64)
    idxval = np.where(half == 1, col - HALF, col).astype(np.int64)

    cnt = np.zeros((NCORES, NB, 2), np.int64)
    np.add.at(cnt, (core, blk, half), 1)
    kL = np.maximum(1, -(-cnt[:, :, 0].max(axis=0) // BLK))  # [NB]
    kH = np.maximum(1, -(-cnt[:, :, 1].max(axis=0) // BLK))  # [NB]

    lt0 = np.concatenate([[0], np.cumsum(kL)])
    TL = int(lt0[-1])
    ht0 = TL + np.concatenate([[0], np.cumsum(kH)])
    T = int(ht0[-1])
    sched = (TL, T - TL,
             [(b, int(lt0[b]), int(kL[b]), int(ht0[b]), int(kH[b]))
              for b in range(NB)])

    idx_flat = np.zeros((NCORES, T * BLK), np.int64)
    lrow_flat = np.zeros((NCORES, T * BLK), np.int64)
    val_flat = np.zeros((NCORES, T * BLK), np.float32)

    order = np.lexsort((col, half, blk, core))
    core_s, blk_s, half_s = core[order], blk[order], half[order]
    idx_s, lrow_s, val_s = idxval[order], lrow[order], val[order]
    key = (core_s * NB + blk_s) * 2 + half_s
    grid = np.arange(NCORES * NB * 2)
    starts = np.searchsorted(key, grid)
    ends = np.searchsorted(key, grid + 1)
    for c in range(NCORES):
        for b in range(NB):
            for h, base in ((0, lt0[b]), (1, ht0[b])):
                g = (c * NB + b) * 2 + h
                s, e = starts[g], ends[g]
                n = e - s
                if n == 0:
                    continue
                off = int(base) * BLK
                idx_flat[c, off:off + n] = idx_s[s:e]
                lrow_flat[c, off:off + n] = lrow_s[s:e]
                val_flat[c, off:off + n] = val_s[s:e]

    # idx wrap: call-local position i = t*128 + p lives at partition i%16,
    # column i//16 = t*8 + p//16 (call starts are tile-aligned, 128%16==0).
    arr = idx_flat.reshape(NCORES, T, 8, 16)           # [c, t, p//16, p%16]
    idx16 = arr.transpose(0, 3, 1, 2).reshape(NCORES, 16, T * 8)
    idx16 = np.ascontiguousarray(
        np.tile(idx16, (1, 8, 1)).astype(np.int16))    # [c, 128, T*8]
    lrowv = np.ascontiguousarray(
        lrow_flat.reshape(NCORES, T, BLK).transpose(0, 2, 1).astype(np.float32))
    valv = np.ascontiguousarray(
        val_flat.reshape(NCORES, T, BLK).transpose(0, 2, 1))
    return idx16, lrow_flat, val_flat, lrowv, valv, sched, T


def _mhost(lrow_flat, val_flat, T, scale=1.0):
    """Host-built M images: M[c][p, t*128 + j] = scale*val of the edge at
    (tile t, slot p) if lrow == j else 0.  [NCORES, 128, T*128] bf16."""
    import ml_dtypes
    M = np.zeros((NCORES, P, T * P), ml_dtypes.bfloat16)
    pos = np.arange(T * BLK)
    t = pos // BLK
    p = pos % BLK
    col = t * P + lrow_flat                            # [NCORES, T*128]
    v = (scale * val_flat).astype(ml_dtypes.bfloat16)
    for c in range(NCORES):
        M[c, p, col[c]] = v[c]
    return M


# ---------------------------------------------------------------- device program
def _build(TA, TN, sched_a, sched_n):
    nc = bass.Bass(num_devices=NCORES, num_swdge_queues=NQ)
    groups = [list(range(NCORES))]

    # ---- external I/O
    xl_in = nc.declare_dram_parameter("xl", [P, R], BF16, isOutput=False)
    a_idx = nc.declare_dram_parameter("a_idx", [P, TA * 8], I16, isOutput=False)
    n_idx = nc.declare_dram_parameter("n_idx", [P, TN * 8], I16, isOutput=False)
    m_a = nc.declare_dram_parameter("m_a", [P, TA * P], BF16, isOutput=False)
    m_nd = nc.declare_dram_parameter("m_nd", [P, TN * P], BF16, isOutput=False)
    m_ndl = nc.declare_dram_parameter("m_ndl", [P, TN * P], BF16, isOutput=False)
    a_lrow = nc.declare_dram_parameter("a_lrow", [P, TA], F32, isOutput=False)
    a_val = nc.declare_dram_parameter("a_val", [P, TA], F32, isOutput=False)
    n_lrow = nc.declare_dram_parameter("n_lrow", [P, TN], F32, isOutput=False)
    n_val = nc.declare_dram_parameter("n_val", [P, TN], F32, isOutput=False)
    n_vall = nc.declare_dram_parameter("n_vall", [P, TN], F32, isOutput=False)
    iota_in = nc.declare_dram_parameter("iota", [P, P], F32, isOutput=False)
    ident_in = nc.declare_dram_parameter("ident", [P, P], BF16, isOutput=False)
    ones1_in = nc.declare_dram_parameter("ones1", [1, P], F32, isOutput=False)
    onesb_in = nc.declare_dram_parameter("onesb", [P, 1], BF16, isOutput=False)
    w1h_in = nc.declare_dram_parameter("w1h", [128, 256], BF16, isOutput=False)
    w10_in = nc.declare_dram_parameter("w10", [128, 128], BF16, isOutput=False)
    w2h_in = nc.declare_dram_parameter("w2h", [384, 256], BF16, isOutput=False)
    w20_in = nc.declare_dram_parameter("w20", [384, 128], BF16, isOutput=False)
    fpw_in = nc.declare_dram_parameter("fpw", [384, 128], BF16, isOutput=False)
    b2h_in = nc.declare_dram_parameter("b2h", [1, 256], F32, isOutput=False)
    b20_in = nc.declare_dram_parameter("b20", [1, 128], F32, isOutput=False)
    fpb_in = nc.declare_dram_parameter("fpb", [1, 128], F32, isOutput=False)
    bng_in = nc.declare_dram_parameter("bng", [1, 384], F32, isOutput=False)
    bnb_in = nc.declare_dram_parameter("bnb", [1, 384], F32, isOutput=False)
    out_ext = nc.declare_dram_parameter("out", [R, 128], F32, isOutput=True)

    with TileContext(nc) as tc:
        with (
            tc.tile_pool(name="consts", bufs=1) as consts,
            tc.tile_pool(name="glp", bufs=10) as glp,
            tc.tile_pool(name="mwp", bufs=8) as mwp,
            tc.tile_pool(name="wk", bufs=4) as wk,
            tc.tile_pool(name="ps", bufs=3, space="PSUM") as ps,
            tc.tile_pool(name="pm", bufs=1, space="PSUM") as pm,
            tc.tile_pool(name="ptr", bufs=2, space="PSUM") as ptrp,
            tc.tile_pool(name="pst", bufs=1, space="PSUM") as pstp,
            tc.tile_pool(name="dram", bufs=1, space="DRAM") as dram,
        ):
            nc.gpsimd.load_library(mlp)

            # num_idxs register cache: Pool has ~46 free registers and
            # to_reg() does not dedupe constants
            _regs = {}

            def nreg(v):
                if v not in _regs:
                    _regs[v] = nc.gpsimd.to_reg(v)
                return _regs[v]

            _qc = [0]

            def next_q():
                q = _qc[0]
                _qc[0] = (q + 1) % NQ
                return q

            # ---------------- DRAM scratch
            y12_loc = dram.tile([R, 256], BF16)
            ta_loc = dram.tile([R, 128], BF16)
            tn_loc = dram.tile([R, 128], BF16)
            z12_loc = dram.tile([R, 256], BF16)
            tpa_loc = dram.tile([R, 128], BF16)
            tpn_loc = dram.tile([R, 128], BF16)
            bn_loc = dram.tile([1, 768], F32)
            ss_dram = dram.tile([6, 128], F32)
            y12_full = dram.tile([N, 256], BF16, addr_space="Shared")
            ta_full = dram.tile([N, 128], BF16, addr_space="Shared")
            tn_full = dram.tile([N, 128], BF16, addr_space="Shared")
            z12_full = dram.tile([N, 256], BF16, addr_space="Shared")
            tpa_full = dram.tile([N, 128], BF16, addr_space="Shared")
            tpn_full = dram.tile([N, 128], BF16, addr_space="Shared")
            bn_full = dram.tile([1, 768], F32, addr_space="Shared")

            # ---------------- constants to SBUF
            def cload(src, shape, dtype):
                t = consts.tile(shape, dtype, name=f"c_{src.name}")
                nc.sync.dma_start(out=t[:], in_=src[:])
                return t

            xl = cload(xl_in, [P, R], BF16)
            identb = cload(ident_in, [P, P], BF16)
            ones1 = cload(ones1_in, [1, P], F32)
            onesb = cload(onesb_in, [P, 1], BF16)
            w1h = cload(w1h_in, [128, 256], BF16)
            w10 = cload(w10_in, [128, 128], BF16)
            b2h = cload(b2h_in, [1, 256], F32)
            b20 = cload(b20_in, [1, 128], F32)
            fpb = cload(fpb_in, [1, 128], F32)
            bng = cload(bng_in, [1, 384], F32)
            bnb = cload(bnb_in, [1, 384], F32)
            w2h_sb, w20_sb, fpw_sb = [], [], []
            for k in range(3):
                t = consts.tile([128, 256], BF16, name=f"w2h{k}")
                nc.sync.dma_start(out=t[:], in_=w2h_in[k * 128:(k + 1) * 128, :])
                w2h_sb.append(t)
                t = consts.tile([128, 128], BF16, name=f"w20{k}")
                nc.sync.dma_start(out=t[:], in_=w20_in[k * 128:(k + 1) * 128, :])
                w20_sb.append(t)
                t = consts.tile([128, 128], BF16, name=f"fpw{k}")
                nc.sync.dma_start(out=t[:], in_=fpw_in[k * 128:(k + 1) * 128, :])
                fpw_sb.append(t)

            aidx = cload(a_idx, [P, TA * 8], I16)
            nidx = cload(n_idx, [P, TN * 8], I16)
            alrow = cload(a_lrow, [P, TA], F32)
            aval = cload(a_val, [P, TA], F32)
            nlrow = cload(n_lrow, [P, TN], F32)
            nval = cload(n_val, [P, TN], F32)
            nvall = cload(n_vall, [P, TN], F32)
            iota = cload(iota_in, [P, P], F32)

            # persistent per-node-block SBUF arrays. Layer-2's three hop
            # slices reuse h1_all's storage: block b's layer-1 features die
            # exactly when its layer-2 values are produced (hop0' is written
            # after the stage-7 transposes read the block; hop1'/hop2' are
            # written in later passes).
            h1_all = consts.tile([P, NB * 384], BF16, name="h1_all")
            sclT = consts.tile([P, 3], F32, name="sclT")
            shfT = consts.tile([P, 3], F32, name="shfT")

            def h2a(b, nb=P):
                return h1_all[:nb, b * 384:b * 384 + 128]

            def h2b(b, nb=P):
                return h1_all[:nb, b * 384 + 128:b * 384 + 256]

            def h2c(b, nb=P):
                return h1_all[:nb, b * 384 + 256:b * 384 + 384]

            def nb_of(b):
                return BLK if b < NB - 1 else R - BLK * (NB - 1)

            # broadcast bias rows to all partitions (b2h, b20*1.5, fpb)
            bcast = {}
            for nm, bsrc, wdt in (("b2h", b2h, 256), ("b20", b20, 128),
                                  ("fpb", fpb, 128)):
                pbx = pm.tile([P, 256], F32, tag="pmm", name=f"pb_{nm}")
                nc.tensor.matmul(out=pbx[:, :wdt], lhsT=ones1[:], rhs=bsrc[:],
                                 start=True, stop=True)
                bt = consts.tile([P, wdt], F32, name=f"bb_{nm}")
                nc.vector.tensor_copy(out=bt[:], in_=pbx[:, :wdt])
                bcast[nm] = bt

            # ---------------- stage 1: local Y12 = X @ [W1;W2]^T (no bias:
            # layer-1 biases cancel in BatchNorm), then AllGather.
            for b in range(NB) if _on('y12') else []:
                nb = nb_of(b)
                py = pm.tile([P, 256], F32, tag="pmm")
                nc.tensor.matmul(out=py[:nb, :], lhsT=xl[:, b * BLK:b * BLK + nb],
                                 rhs=w1h[:], start=True, stop=True)
                yb = wk.tile([P, 256], BF16, tag="yb")
                nc.scalar.activation(out=yb[:nb, :], in_=py[:nb, :], func=AF.Copy)
                nc.scalar.dma_start(out=y12_loc[b * BLK:b * BLK + nb, :],
                                    in_=yb[:nb, :])
            if _on('y12'):
                nc.gpsimd.collective_compute(
                    "AllGather", AL.bypass, replica_groups=groups,
                    ins=[y12_loc[:]], outs=[y12_full[:]])

            # hop0 while the AllGather runs: h1 hop0 = 1.5 * x @ W0^T
            # (w10 pre-scaled by 1.5 on host; bias cancels in BN)
            for b in range(NB) if _on('hop0') else []:
                nb = nb_of(b)
                p0 = pm.tile([P, 256], F32, tag="pmm")
                nc.tensor.matmul(out=p0[:nb, :128], lhsT=xl[:, b * BLK:b * BLK + nb],
                                 rhs=w10[:], start=True, stop=True)
                nc.scalar.activation(out=h1_all[:nb, b * 384:b * 384 + 128],
                                     in_=p0[:nb, :128], func=AF.Copy)

            # ---------------- batched-gather SpMM pass: L phase over the
            # low table half, then H phase over the high half (int16 gather
            # indices). Gather calls cover WIN-tile windows, round-robin
            # across the 4 SWDGE queues; the matching M window streams in
            # via HWDGE. Each block accumulates one PSUM per phase; the H
            # drain combines with the L result.
            tstage = consts.tile([P, NB * 128], BF16, name="tstage")

            def spmm_phase(pfx, table_half, idxT, mdram, lrowT, valT,
                           t0, ntiles, binfo, elem, drain):
                # software-pipelined: windows (gather + M) issue PREF ahead
                # of the consuming matmuls so every engine stream stays
                # interleaved. M source alternates: VDVE_OF windows built
                # on VectorE (per-tile is_equal/mult from lrow/val), the
                # rest streamed from the host-built DRAM image via HWDGE.
                nwin = (ntiles + WIN - 1) // WIN
                wins = [None] * nwin
                mwins = [None] * nwin

                def issue(wi):
                    w0 = wi * WIN
                    n = min(WIN, ntiles - w0)
                    g = glp.tile([P, WIN * elem], BF16, tag="gw",
                                 name=f"{pfx}g{w0}")
                    nc.gpsimd.dma_gather(
                        out_ap=g[:, :n * elem].rearrange(
                            "p (t e) -> p t e", e=elem),
                        in_ap=table_half,
                        idxs_ap=idxT[:, (t0 + w0) * 8:(t0 + w0 + n) * 8],
                        num_idxs=n * BLK, num_idxs_reg=nreg(n * BLK),
                        elem_size=elem, queue_num=next_q())
                    wins[wi] = g
                    mw = mwp.tile([P, WIN * P], BF16, tag="mw",
                                  name=f"{pfx}m{w0}")
                    if wi % 5 < VDVE_OF:
                        for j in range(n):
                            t = t0 + w0 + j
                            nc.vector.tensor_scalar(
                                out=mw[:, j * P:(j + 1) * P], in0=iota[:],
                                scalar1=lrowT[:, t:t + 1],
                                scalar2=valT[:, t:t + 1],
                                op0=AL.is_equal, op1=AL.mult)
                    else:
                        eng = nc.sync if wi % 2 == 0 else nc.scalar
                        eng.dma_start(
                            out=mw[:, :n * P],
                            in_=mdram[:, (t0 + w0) * P:(t0 + w0 + n) * P])
                    mwins[wi] = mw

                PREF = 7
                issued = 0
                for (b, bt0, k) in binfo:
                    need = (bt0 - t0 + k + WIN - 1) // WIN
                    while issued < min(need + PREF, nwin):
                        issue(issued)
                        issued += 1
                    psum = ps.tile([P, elem], F32, tag="sp", name=f"{pfx}ps{b}")
                    for i in range(k):
                        t = bt0 + i
                        lt = t - t0
                        g = wins[lt // WIN]
                        mw = mwins[lt // WIN]
                        s = lt % WIN
                        nc.tensor.matmul(
                            out=psum[:], lhsT=mw[:, s * P:(s + 1) * P],
                            rhs=g[:, s * elem:(s + 1) * elem],
                            start=(i == 0), stop=(i == k - 1))
                    drain(b, psum)

            def spmm_pass(pfx, table, idxT, mdram, lrowT, valT, sched,
                          elem, drainL, drainH):
                TL, TH, blocks = sched
                spmm_phase(pfx + "L", table[0:HALF, :], idxT, mdram, lrowT,
                           valT, 0, TL,
                           [(b, lt0, kL) for (b, lt0, kL, _, _) in blocks],
                           elem, drainL)
                spmm_phase(pfx + "H", table[HALF:N, :], idxT, mdram, lrowT,
                           valT, TL, TH,
                           [(b, ht0, kH) for (b, _, _, ht0, kH) in blocks],
                           elem, drainH)

            # ---------------- layer 1, first application (hop1 + T tables)
            def mk_first_app(hslice, lam_first, t_loc):
                # hslice(b) -> target AP for the hop-1 slice; lam_first:
                # None for the A pass (copy), LAM for the ND pass (l-add)
                def dL(b, p):
                    sl = hslice(b)
                    if lam_first is None:
                        nc.vector.tensor_copy(out=sl, in_=p[:, 0:128])
                    else:
                        nc.vector.scalar_tensor_tensor(
                            out=sl, in0=p[:, 0:128], scalar=lam_first, in1=sl,
                            op0=AL.mult, op1=AL.add)
                    nc.scalar.activation(out=tstage[:, b * 128:(b + 1) * 128],
                                         in_=p[:, 128:256], func=AF.Copy)

                def dH(b, p):
                    nb = nb_of(b)
                    sl = hslice(b)
                    if lam_first is None:
                        nc.vector.tensor_tensor(out=sl, in0=p[:, 0:128],
                                                in1=sl, op=AL.add)
                    else:
                        nc.vector.scalar_tensor_tensor(
                            out=sl, in0=p[:, 0:128], scalar=lam_first, in1=sl,
                            op0=AL.mult, op1=AL.add)
                    tsb = wk.tile([P, 128], BF16, tag="tsb")
                    nc.vector.tensor_tensor(
                        out=tsb[:], in0=p[:, 128:256],
                        in1=tstage[:, b * 128:(b + 1) * 128], op=AL.add)
                    nc.sync.dma_start(out=t_loc[b * BLK:b * BLK + nb, :],
                                      in_=tsb[:nb, :])
                return dL, dH

            def h1_hop1(b, nb=P):
                return h1_all[:nb, b * 384 + 128:b * 384 + 256]

            def h1_hop2(b, nb=P):
                return h1_all[:nb, b * 384 + 256:b * 384 + 384]

            if _on('a1'):
                dL, dH = mk_first_app(h1_hop1, None, ta_loc)
                spmm_pass("a1", y12_full, aidx, m_a, alrow, aval, sched_a,
                          256, dL, dH)
                nc.gpsimd.collective_compute(
                    "AllGather", AL.bypass, replica_groups=groups,
                    ins=[ta_loc[:]], outs=[ta_full[:]])
            if _on('n1'):
                dL, dH = mk_first_app(h1_hop1, LAM, tn_loc)
                spmm_pass("n1", y12_full, nidx, m_nd, nlrow, nval, sched_n,
                          256, dL, dH)
                nc.gpsimd.collective_compute(
                    "AllGather", AL.bypass, replica_groups=groups,
                    ins=[tn_loc[:]], outs=[tn_full[:]])

            # ---------------- layer 1, second application (hop2) + BN stats
            # ND second applications use the lambda-prescaled M image, so
            # every drain is a plain copy/add.
            def mk_second_app(hslice, init, extra=None):
                def dL(b, p):
                    sl = hslice(b)
                    if init:
                        nc.vector.tensor_copy(out=sl, in_=p[:, 0:128])
                    else:
                        nc.vector.tensor_tensor(out=sl, in0=p[:, 0:128],
                                                in1=sl, op=AL.add)

                def dH(b, p):
                    sl = hslice(b)
                    nc.vector.tensor_tensor(out=sl, in0=p[:, 0:128],
                                            in1=sl, op=AL.add)
                    if extra is not None:
                        extra(b)
                return dL, dH

            if _on('a2'):
                dL, dH = mk_second_app(h1_hop2, True)
                spmm_pass("a2", ta_full, aidx, m_a, alrow, aval, sched_a,
                          128, dL, dH)

            pst = pstp.tile([1, 384], F32, tag="pst", name="pst")
            psq = pstp.tile([1, 384], F32, tag="psq", name="psq")

            def stats_extra(b):
                nb = nb_of(b)
                hsl = h1_all[:, b * 384:b * 384 + 384]
                sq = wk.tile([P, 384], BF16, tag="sq")
                nc.scalar.activation(out=sq[:nb, :], in_=hsl[:nb],
                                     func=AF.Square)
                nc.tensor.matmul(out=pst[:], lhsT=onesb[:nb, :], rhs=hsl[:nb],
                                 start=(b == 0), stop=(b == NB - 1))
                nc.tensor.matmul(out=psq[:], lhsT=onesb[:nb, :], rhs=sq[:nb, :],
                                 start=(b == 0), stop=(b == NB - 1))

            if _on('n2'):
                dL, dH = mk_second_app(h1_hop2, False, extra=stats_extra)
                spmm_pass("n2", tn_full, nidx, m_ndl, nlrow, nvall, sched_n,
                          128, dL, dH)

            # ---------------- BN finalize (allreduce + feature-major params)
            if _on('bn'):
                stats = wk.tile([1, 768], F32, tag="bnst", bufs=1)
                nc.vector.tensor_copy(out=stats[:, 0:384], in_=pst[:])
                nc.vector.tensor_copy(out=stats[:, 384:768], in_=psq[:])
                nc.sync.dma_start(out=bn_loc[:], in_=stats[:])
                nc.gpsimd.collective_compute(
                    "AllReduce", AL.add, replica_groups=groups,
                    ins=[bn_loc[:]], outs=[bn_full[:]])
                bnr = wk.tile([1, 768], F32, tag="bnr", bufs=1)
                nc.sync.dma_start(out=bnr[:], in_=bn_full[:])
                mean = wk.tile([1, 384], F32, tag="bn1", bufs=1)
                var = wk.tile([1, 384], F32, tag="bn2", bufs=1)
                tmp = wk.tile([1, 384], F32, tag="bn3", bufs=1)
                sshf = wk.tile([1, 768], F32, tag="bn4", bufs=1)
                nc.vector.tensor_scalar(out=mean[:], in0=bnr[:, 0:384],
                                        scalar1=1.0 / N, scalar2=None, op0=AL.mult)
                nc.vector.tensor_scalar(out=var[:], in0=bnr[:, 384:768],
                                        scalar1=1.0 / N, scalar2=None, op0=AL.mult)
                nc.vector.tensor_tensor(out=tmp[:], in0=mean[:], in1=mean[:], op=AL.mult)
                nc.vector.tensor_tensor(out=var[:], in0=var[:], in1=tmp[:], op=AL.subtract)
                nc.vector.tensor_scalar(out=var[:], in0=var[:], scalar1=EPS,
                                        scalar2=None, op0=AL.add)
                nc.scalar.sqrt(out=var[:], in_=var[:])
                nc.vector.reciprocal(out=var[:], in_=var[:])
                nc.vector.tensor_tensor(out=sshf[:, 0:384], in0=bng[:], in1=var[:],
                                        op=AL.mult)
                nc.vector.tensor_tensor(out=tmp[:], in0=mean[:], in1=sshf[:, 0:384],
                                        op=AL.mult)
                nc.vector.tensor_tensor(out=sshf[:, 384:768], in0=bnb[:], in1=tmp[:],
                                        op=AL.subtract)
                nc.sync.dma_start(out=ss_dram[:], in_=sshf[:])
                for k in range(3):
                    nc.sync.dma_start(out=sclT[:, k:k + 1], in_=ss_dram[k:k + 1, :])
                    nc.sync.dma_start(out=shfT[:, k:k + 1], in_=ss_dram[k + 3:k + 4, :])

            # ---------------- layer-2 dense: transpose, fused BN+relu (ACT),
            # linears; z12 local then AllGather
            for b in range(NB) if _on('dense') else []:
                nb = nb_of(b)
                hbT = wk.tile([P, 384], BF16, tag="hbT")
                for k in range(3):
                    pt = ptrp.tile([P, 128], BF16, tag="pt")
                    nc.tensor.transpose(
                        out=pt[:, :nb],
                        in_=h1_all[:nb, b * 384 + k * 128:b * 384 + (k + 1) * 128],
                        identity=identb[:nb, :nb])
                    nc.scalar.activation(
                        out=hbT[:, k * 128:k * 128 + nb], in_=pt[:, :nb],
                        func=AF.Relu, scale=sclT[:, k:k + 1], bias=shfT[:, k:k + 1])
                pz = pm.tile([P, 256], F32, tag="pmm")
                for k in range(3):
                    nc.tensor.matmul(out=pz[:nb, :],
                                     lhsT=hbT[:, k * 128:k * 128 + nb],
                                     rhs=w2h_sb[k][:], start=(k == 0), stop=(k == 2))
                zt = wk.tile([P, 256], BF16, tag="yb")
                nc.vector.tensor_tensor(out=zt[:nb, :], in0=pz[:nb, :],
                                        in1=bcast["b2h"][:nb, :], op=AL.add)
                nc.sync.dma_start(out=z12_loc[b * BLK:b * BLK + nb, :], in_=zt[:nb, :])
                p0 = pm.tile([P, 256], F32, tag="pmm")
                for k in range(3):
                    nc.tensor.matmul(out=p0[:nb, :128],
                                     lhsT=hbT[:, k * 128:k * 128 + nb],
                                     rhs=w20_sb[k][:], start=(k == 0), stop=(k == 2))
                nc.vector.tensor_tensor(out=h2a(b, nb),
                                        in0=p0[:nb, :128], in1=bcast["b20"][:nb, :],
                                        op=AL.add)
            if _on('dense'):
                nc.gpsimd.collective_compute(
                    "AllGather", AL.bypass, replica_groups=groups,
                    ins=[z12_loc[:]], outs=[z12_full[:]])

            # ---------------- layer 2, first + second applications
            if _on('a3'):
                dL, dH = mk_first_app(h2b, None, tpa_loc)
                spmm_pass("a3", z12_full, aidx, m_a, alrow, aval, sched_a,
                          256, dL, dH)
                nc.gpsimd.collective_compute(
                    "AllGather", AL.bypass, replica_groups=groups,
                    ins=[tpa_loc[:]], outs=[tpa_full[:]])
            if _on('n3'):
                dL, dH = mk_first_app(h2b, LAM, tpn_loc)
                spmm_pass("n3", z12_full, nidx, m_nd, nlrow, nval, sched_n,
                          256, dL, dH)
                nc.gpsimd.collective_compute(
                    "AllGather", AL.bypass, replica_groups=groups,
                    ins=[tpn_loc[:]], outs=[tpn_full[:]])

            if _on('a4'):
                dL, dH = mk_second_app(h2c, True)
                spmm_pass("a4", tpa_full, aidx, m_a, alrow, aval, sched_a,
                          128, dL, dH)

            def final_extra(b):
                nb = nb_of(b)
                hbT = wk.tile([P, 384], BF16, tag="hbT")
                for k, hsrc in enumerate((h2a, h2b, h2c)):
                    pt = ptrp.tile([P, 128], BF16, tag="pt")
                    nc.tensor.transpose(
                        out=pt[:, :nb], in_=hsrc(b, nb),
                        identity=identb[:nb, :nb])
                    nc.scalar.activation(out=hbT[:, k * 128:k * 128 + nb],
                                         in_=pt[:, :nb], func=AF.Copy)
                po = pm.tile([P, 256], F32, tag="pmm")
                for k in range(3):
                    nc.tensor.matmul(out=po[:nb, :128],
                                     lhsT=hbT[:, k * 128:k * 128 + nb],
                                     rhs=fpw_sb[k][:], start=(k == 0),
                                     stop=(k == 2))
                osb = wk.tile([P, 128], F32, tag="osb")
                nc.vector.tensor_tensor(out=osb[:nb, :], in0=po[:nb, :128],
                                        in1=bcast["fpb"][:nb, :], op=AL.add)
                nc.sync.dma_start(out=out_ext[b * BLK:b * BLK + nb, :],
                                  in_=osb[:nb, :])

            if _on('n4'):
                dL, dH = mk_second_app(h2c, False, extra=final_extra)
                spmm_pass("n4", tpn_full, nidx, m_ndl, nlrow, nvall, sched_n,
                          128, dL, dH)
            if STOP_AFTER is not None:
                dz = wk.tile([P, 128], F32, tag="osb")
                nc.vector.memset(dz[:], 0.0)
                for b in range(NB):
                    nb = nb_of(b)
                    nc.sync.dma_start(out=out_ext[b * BLK:b * BLK + nb, :],
                                      in_=dz[:nb, :])

    return nc


def _make(x, val, nd_val,
          l1_W0, l1_b0, l1_W1, l1_b1, l1_W2, l1_b2,
          l2_W0, l2_b0, l2_W1, l2_b1, l2_W2, l2_b2,
          bn_gamma, bn_beta, fp_W, fp_b,
          row, col, nd_row, nd_col):
    import ml_dtypes
    x = np.asarray(x, np.float32)
    row = np.asarray(row, np.int64); col = np.asarray(col, np.int64)
    val = np.asarray(val, np.float32)
    nd_row = np.asarray(nd_row, np.int64); nd_col = np.asarray(nd_col, np.int64)
    nd_val = np.asarray(nd_val, np.float32)

    a_i, a_lr, a_vf, a_lrv, a_vv, sched_a, TA = _prep_adj(row, col, val)
    n_i, n_lr, n_vf, n_lrv, n_vv, sched_n, TN = _prep_adj(nd_row, nd_col, nd_val)
    m_a = _mhost(a_lr, a_vf, TA)
    m_nd = _mhost(n_lr, n_vf, TN)
    m_ndl = _mhost(n_lr, n_vf, TN, scale=LAM)

    bf = ml_dtypes.bfloat16
    xt = np.ascontiguousarray(x.T).astype(bf)                # [128, N]
    ident = np.eye(P, dtype=np.float32).astype(bf)
    ones1 = np.ones((1, P), np.float32)
    onesb = np.ones((P, 1), bf)
    w1h = np.ascontiguousarray(
        np.concatenate([l1_W1, l1_W2], 0).T).astype(bf)      # [128, 256]
    w10 = np.ascontiguousarray(
        (1.0 + LAM) * np.asarray(l1_W0, np.float32).T).astype(bf)
    w2h = np.ascontiguousarray(
        np.concatenate([l2_W1, l2_W2], 0).T).astype(bf)      # [384, 256]
    w20 = np.ascontiguousarray(
        (1.0 + LAM) * np.asarray(l2_W0, np.float32).T).astype(bf)
    fpw = np.ascontiguousarray(np.asarray(fp_W, np.float32).T).astype(bf)
    b2h = np.concatenate([l2_b1, l2_b2])[None, :].astype(np.float32)
    b20 = ((1.0 + LAM) * np.asarray(l2_b0))[None, :].astype(np.float32)
    fpb = np.asarray(fp_b)[None, :].astype(np.float32)
    bng = np.asarray(bn_gamma)[None, :].astype(np.float32)
    bnb = np.asarray(bn_beta)[None, :].astype(np.float32)

    nc = _build(TA, TN, sched_a, sched_n)
    # raw Bass skips Bacc's extended-inst codegen pass; without it the NEFF
    # compiler sees empty .instr on ISA subclasses -> "ISA wrong length"
    mybir.codegen_inst_isa_subclasses(nc)
    if FIX_WAITS:
        fix_waits(nc)

    in_maps = []
    for c in range(NCORES):
        in_maps.append({
            "xl": np.ascontiguousarray(xt[:, c * R:(c + 1) * R]),
            "a_idx": a_i[c], "n_idx": n_i[c],
            "m_a": m_a[c], "m_nd": m_nd[c], "m_ndl": m_ndl[c],
            "a_lrow": a_lrv[c], "a_val": a_vv[c],
            "n_lrow": n_lrv[c], "n_val": n_vv[c],
            "n_vall": (LAM * n_vv[c]),
            "iota": np.tile(np.arange(P, dtype=np.float32)[None, :], (P, 1)),
            "ident": ident, "ones1": ones1, "onesb": onesb,
            "w1h": w1h, "w10": w10, "w2h": w2h, "w20": w20, "fpw": fpw,
            "b2h": b2h, "b20": b20, "fpb": fpb, "bng": bng, "bnb": bnb,
        })
    return nc, in_maps


def kernel(**inputs):
    nc, in_maps = _make(**inputs)
    res = run_bass_kernel_spmd(nc, in_maps, list(range(NCORES)), trace=TRACE)
    LAST_RESULT["res"] = res
    out = np.concatenate([res.results[c]["out"] for c in range(NCORES)], axis=0)
    return out
